# revision 94
# baseline (speedup 1.0000x reference)
"""Trainium2 Bass kernel for a GPT-2 style transformer block (B=4, S=2048, D=768).

Sharding (8 NeuronCores, one SPMD program):
  core c = (b, p): b = c // 2 (batch), p = c % 2 (tensor-parallel rank).
  - Attention is head-split: p=0 owns heads 0..5, p=1 owns heads 6..11,
    each over the FULL sequence of its batch.
  - c_attn / c_proj are computed only for the owned heads; the partial
    c_proj outputs are summed + token-scattered with a ReduceScatter over
    core pairs [[0,1],[2,3],[4,5],[6,7]] (bf16 payload).
  - LN1 / FFN / LN2 are token-split: p owns tokens [p*1024, (p+1)*1024).

All matmuls run in bf16 (fp32 PSUM accumulation). All DRAM tensors are
pre-shuffled host-side into partition-major [128, ...] layouts so DMAs are
contiguous per partition. Scores matmuls pack 2 heads via PE row tiling
(K=64 each); LN mean/var matmuls pack via PE column tiling. Softmax
denominators come free from a ones-column appended to V (M=65 AV matmuls);
reciprocals use the fast custom-DVE approximation.
"""

import numpy as np
import ml_dtypes

import concourse.bass as bass
import concourse.mybir as mybir
import concourse.tile as tile
from concourse import bacc
from concourse.bass_utils import run_bass_kernel_spmd

# ---------------------------------------------------------------- constants
B = 4
S = 2048
D = 768
H = 12
DH = 64
F = 3072
EPS = 1e-5

N_CORES = 8
HL = H // 2            # heads per core (6)
FH = HL * DH           # per-core attention feature width (384)
KC = D // 128          # contraction chunks over D (6)
QC = FH // 128         # feature chunks for per-core q or k (3)
FC = F // 128          # fc feature chunks (24)
QT = 512               # attention q-tile width
GQ = S // QT           # q tiles over full sequence (4)
DQT = QT // 128        # k-blocks per q tile width (4)
NKT = S // 128         # k-blocks over full sequence (16)
HALF = S // 2          # tokens owned per core for FFN/LN (1024)
TT = HALF // QT        # 512-token tiles per half (2)

FP = mybir.dt.float32
FPR = mybir.dt.float32r
BF = mybir.dt.bfloat16
F8 = mybir.dt.float8e4
DR = mybir.MatmulPerfMode.DoubleRow
FS = 16.0                # host-side scale on fp8 fc weights (dynamic range)

AF = mybir.ActivationFunctionType
OP = mybir.AluOpType


def _recip_fast(nc, out, in_):
    """out = approx 1/in_ (custom DVE op); out dtype may be bf16 (rounds
    on writeback), in_ must be fp32/PSUM-fp32 (bit-trick seed needs fp32
    layout)."""
    from concourse.dve_ops import RECIP_APPROX_FAST_CONSTS, RECIPROCAL_APPROX_FAST

    c = RECIP_APPROX_FAST_CONSTS
    return nc.vector._custom_dve(
        RECIPROCAL_APPROX_FAST, out=out, in0=in_, s0=c["s0"], s1=c["s1"],
        imm2=c["imm2"],
    )


def emit_block(tc, outs, ins):
    nc = tc.nc
    with nc.allow_low_precision(reason="bf16 matmul pipeline by design"):
        _emit_block_inner(tc, outs, ins)


def _emit_block_inner(tc, outs, ins):
    nc = tc.nc
    outT = outs["outT"]

    xpm = ins["xpm"]             # [128, KC, S] bf16
    xh = ins["xh"]               # [128, KC, HALF] bf16
    wqk = ins["wqk"]             # [128, KC, 2*FH] bf16
    bqk = ins["bqk"]             # [128, 2*QC] fp32
    wv = ins["wv"]               # [128, KC, FH] bf16
    bv = ins["bv"]               # [128, HL, DH] bf16 (broadcast materialized)
    wpr = ins["wpr"]             # [128, QC, D] bf16
    bpr = ins["bpr"]             # [128, KC] fp32
    gb1 = ins["gb1"]             # [128, KC] fp32
    gb2 = ins["gb2"]             # [128, KC] fp32
    wfc = ins["wfc"]             # [128, KC, F] bf16
    bfc = ins["bfc"]             # [128, FC] fp32
    wfc2 = ins["wfc2"]           # [128, FC, D] bf16
    bfc2 = ins["bfc2"]           # [128, KC] fp32
    mask = ins["mask"]           # [128, DQT, QT] bf16: mask[p, r, t] = (128r+p) <= t

    with (
        tc.tile_pool(name="const", bufs=1) as const,
        tc.tile_pool(name="dram", bufs=1, space="DRAM") as dram,
    ):
        # ---------------- constants
        # phase-1 consts go on the scalar HWDGE queue (cheap fixed cost);
        # FFN consts are deferred to ffn_loads() -- putting them on the
        # gpsimd SWDGE queue up front costs ~2us fixed EACH and delays the
        # attention weight loads behind them
        mask_sb = const.tile([128, DQT, QT], BF)
        bqk_sb = const.tile([128, 2 * QC], FP)
        nc.scalar.dma_start(bqk_sb, bqk)
        bv_sb = const.tile([128, HL, DH], BF)
        nc.scalar.dma_start(bv_sb, bv)
        bpr_sb = const.tile([128, KC], FP)
        gb1_sb = const.tile([128, KC], FP)
        gb2_sb = const.tile([128, KC], FP)
        bfc_sb = const.tile([128, FC], FP)
        bfc2_sb = const.tile([128, KC], FP)
        ones128 = const.tile([128, 1], BF)
        nc.vector.memset(ones128, 1.0)
        ones_row = const.tile([1, 128], BF)
        nc.vector.memset(ones_row, 1.0)
        eps_sb = const.tile([1, 1], FP)
        nc.vector.memset(eps_sb, EPS)
        # mask load is issued later (gpsimd queue, behind the attention
        # weights) so phase-1's x/wqk loads aren't delayed behind it

        # one DRAM tile per chunk so chunk-0 consumers don't falsely
        # depend on chunk-1's collective (dep tracking is per-tile)
        a_bounce = [
            dram.tile([2, 128, KC, QT], BF, tag=f"ab{c}", name=f"ab{c}")
            for c in range(TT)
        ]
        rs_out = [
            dram.tile([128, KC, QT], BF, tag=f"rs{c}", name=f"rs{c}")
            for c in range(TT)
        ]

        with tc.tile_pool(name="ffw", bufs=1) as ffw:
            # fc1 weights fp8 (x16 host-scaled, DoubleRow contracts 256
            # features per MM); fc2 stays bf16 -- quantizing both matmuls
            # pushes the output past the accuracy budget
            wfc_sb = ffw.tile([128, KC, F], F8)
            wfc2_sb = ffw.tile([128, FC, D], BF)

            with tc.tile_pool(name="nt", bufs=1) as npool:
                nT_sb = npool.tile([128, KC, HALF], BF)
                n8_sb = npool.tile([128, KC, HALF], F8)
                xh_sb = npool.tile([128, KC, HALF], BF)
                rs_sbs = [
                    npool.tile([128, KC, QT], BF, tag=f"rsb{i}", name=f"rsb{i}")
                    for i in range(TT)
                ]
                def ffn_loads():
                    nc.scalar.dma_start(bpr_sb, ins["bpr"])
                    nc.scalar.dma_start(gb1_sb, ins["gb1"])
                    nc.scalar.dma_start(gb2_sb, ins["gb2"])
                    nc.scalar.dma_start(bfc_sb, ins["bfc"])
                    nc.scalar.dma_start(bfc2_sb, ins["bfc2"])
                    nc.scalar.dma_start(xh_sb, xh)
                    nc.scalar.dma_start(wfc_sb, ins["wfc"])
                    nc.scalar.dma_start(wfc2_sb, ins["wfc2"])

                _emit_qkv_attention(
                    tc, ins, a_bounce, rs_out, rs_sbs, mask_sb, bqk_sb,
                    bv_sb, ones_row, ffn_loads,
                )

                # ====== phases 4+5: LN1 -> FFN -> LN2 per 512-token chunk ==
                _emit_ffn(
                    tc, outT, nT_sb, n8_sb, xh_sb, rs_sbs, wfc_sb, wfc2_sb,
                    bpr_sb, gb1_sb, gb2_sb, bfc_sb, bfc2_sb, ones128, eps_sb,
                    ones_row,
                )


def _emit_qkv_attention(tc, ins, a_bounce, rs_out, rs_sbs, mask_sb, bqk_sb,
                        bv_sb, ones_row, ffn_loads):
    nc = tc.nc
    xpm = ins["xpm"]
    wqk = ins["wqk"]
    wv = ins["wv"]
    wpr = ins["wpr"]

    with tc.tile_pool(name="kqv", bufs=1) as kqv:
        # persistent attention activations (bf16)
        kT_sb = kqv.tile([128, QC, S], BF)            # k, feature-major
        qT_sb = kqv.tile([128, QC, S], BF)            # q, feature-major
        v_sb = kqv.tile([128, NKT, HL, DH + 1], BF)   # v token-major + ones
        nc.vector.memset(v_sb[:, :, :, DH : DH + 1], 1.0)

        # ================ phase 1: qkv projections =====================
        with (
            tc.tile_pool(name="p1", bufs=2) as p1,
            tc.tile_pool(name="p1w", bufs=1) as p1w,
            tc.tile_pool(name="psqk", bufs=6, space="PSUM") as psqk,
            tc.tile_pool(name="psv", bufs=2, space="PSUM") as psv,
        ):
            # all phase-1 loads issued up front as few, large DMAs (HWDGE
            # fixed cost is ~0.6us EACH): wqk split across the two HWDGE
            # queues, x split 3 ways (sync/scalar/gpsimd) so the first-MM
            # gate isn't one queue's serial drain; mask goes LAST on
            # gpsimd (first needed only when attention starts). FFN
            # weights only start after these (ffn_loads).
            KH = KC // 2
            wqk_sb = p1w.tile([128, KC, 2 * FH], BF)
            nc.sync.dma_start(wqk_sb[:, 0:KH, :], wqk[:, 0:KH, :])
            nc.scalar.dma_start(wqk_sb[:, KH:, :], wqk[:, KH:, :])
            wv_sb = p1w.tile([128, KC, FH], BF)
            nc.gpsimd.dma_start(wv_sb, wv)
            x_ts = []
            for half in range(2):
                t0 = half * HALF
                x_t = p1.tile([128, KC, HALF], BF, tag="xT")
                x_ts.append(x_t)
                nc.sync.dma_start(
                    x_t[:, 0:2, :], xpm[:, 0:2, t0 : t0 + HALF]
                )
                nc.scalar.dma_start(
                    x_t[:, 2:4, :], xpm[:, 2:4, t0 : t0 + HALF]
                )
                nc.gpsimd.dma_start(
                    x_t[:, 4:6, :], xpm[:, 4:6, t0 : t0 + HALF]
                )
            nc.gpsimd.dma_start(mask_sb, ins["mask"])

            for half in range(2):
                t0 = half * HALF
                x_t = x_ts[half]
                # q / k feature-major: out[feat_chunk, tokens]
                for fc in range(2 * QC):
                    for ttt in range(TT):
                        ps = psqk.tile([128, QT], FP)
                        for k in range(KC):
                            nc.tensor.matmul(
                                ps,
                                lhsT=wqk_sb[:, k, 128 * fc : 128 * fc + 128],
                                rhs=x_t[:, k, QT * ttt : QT * ttt + QT],
                                start=(k == 0),
                                stop=(k == KC - 1),
                            )
                        dst = qT_sb if fc < QC else kT_sb
                        cc = fc if fc < QC else fc - QC
                        nc.vector.tensor_scalar_add(
                            dst[:, cc, t0 + QT * ttt : t0 + QT * ttt + QT],
                            ps,
                            bqk_sb[:, fc : fc + 1],
                        )
                # v token-major: out[token_chunk, v features]
                for tcc in range(HALF // 128):
                    ps = psv.tile([128, FH], FP)
                    for k in range(KC):
                        nc.tensor.matmul(
                            ps,
                            lhsT=x_t[:, k, 128 * tcc : 128 * tcc + 128],
                            rhs=wv_sb[:, k, :],
                            start=(k == 0),
                            stop=(k == KC - 1),
                        )
                    tok = half * (HALF // 128) + tcc
                    nc.vector.tensor_add(
                        v_sb[:, tok, :, 0:DH],
                        ps.rearrange("p (h d) -> p h d", h=HL),
                        bv_sb,
                    )

        # FFN weights + residual stream: issued only now so phase-1's
        # x/wqk loads aren't stuck behind 9.4MB of FFN weights
        ffn_loads()
        # ============ phase 2+3: attention, normalize, c_proj ==========
        _emit_attention(
            tc, a_bounce, rs_out, rs_sbs, kT_sb, qT_sb, v_sb, wpr, mask_sb,
            ones_row,
        )


def _emit_attention(tc, a_bounce, rs_out, rs_sbs, kT_sb, qT_sb, v_sb, wpr,
                    mask_sb, ones_row):
    nc = tc.nc
    with (
        tc.tile_pool(name="att", bufs=1) as att,
        tc.tile_pool(name="attR", bufs=2) as attR,
        tc.tile_pool(name="pss", bufs=2, space="PSUM") as pss,
        tc.tile_pool(name="psav", bufs=1, space="PSUM") as psav,
        tc.tile_pool(name="pscp", bufs=2, space="PSUM") as pscp,
    ):
        aT_sb = att.tile([128, QC, S], BF)    # attention out, feature-major
        wpr_sb = att.tile([128, QC, D], BF)
        nc.sync.dma_start(wpr_sb, wpr)
        # manual exp ring: masked-out columns of diagonal blocks are
        # never re-exp'd; the mask multiply zeroes them against stale
        # (finite) data, seeded by this one-time memset.
        exm = att.tile([128, 3, 2, QT], BF)
        nc.vector.memset(exm, 0.0)

        # the per-g tail (normalize + c_proj + ReduceScatter) is split into
        # ~10 small pieces drained ONE PER j-ITERATION of the next g's
        # attention loops: each piece's PE/DVE work hides in the slack
        # under the exp stream instead of stalling softmax ~14us per g
        tail_q = []

        def drain_piece():
            if tail_q:
                tail_q.pop(0)()

        for gi, g in enumerate([0, 2, 1, 3]):
            q0 = g * QT
            nkt = DQT * (g + 1)
            rec_sb = attR.tile([1, HL * QT], BF, tag="rec")
            def scores(m, j):
                ps = pss.tile([128, 2, QT], FP, tag="ss", name="ps")
                jd = j - DQT * g
                c0 = 128 * jd if jd > 0 else 0
                # 2 heads row-tiled (K=64 each) -> concurrent MMs;
                # diagonal blocks stream only the causally-live cols
                for i in range(2):
                    hr = i * 64
                    nc.tensor.matmul(
                        ps[:, i, c0:],
                        lhsT=kT_sb[
                            hr : hr + 64, m, 128 * j : 128 * j + 128
                        ],
                        rhs=qT_sb[hr : hr + 64, m, q0 + c0 : q0 + QT],
                        start=True,
                        stop=True,
                    )
                return ps

            ps = scores(0, 0)
            for m in range(QC):  # head pairs (2m, 2m+1)
                den_sb = attR.tile([1, 2, QT], FP, tag="den")
                pavs = [
                    psav.tile(
                        [DH + 1, QT], FP, tag=f"pav{i}", name=f"pav{i}"
                    )
                    for i in range(2)
                ]
                for j in range(nkt):
                    jd = j - DQT * g
                    c0 = 128 * jd if jd > 0 else 0
                    ex = exm[:, j % 3, :, :]
                    nc.scalar.activation(
                        out=ex[:, :, c0:],
                        in_=ps[:, :, c0:],
                        func=AF.Exp,
                    )
                    # software pipeline: the NEXT scores matmuls (next j,
                    # or the next head-pair's j=0) are emitted BEFORE this
                    # j's AV matmuls. The PE queue is strict FIFO and AV_j
                    # waits on exp_j -- anything behind AV_j serializes at
                    # (exp+AV+scores) per step instead of running exp-bound
                    # with the PE work hidden under it. Carrying the
                    # lookahead ACROSS head-pair boundaries removes the
                    # ~2-4us softmax stall at each of the 12 boundaries.
                    if j + 1 < nkt:
                        ps = scores(m, j + 1)
                    elif m + 1 < QC:
                        ps = scores(m + 1, 0)
                    if jd >= 0:
                        for i in range(2):
                            nc.vector.tensor_mul(
                                ex[:, i, :], ex[:, i, :], mask_sb[:, jd, :]
                            )
                    for i in range(2):
                        nc.tensor.matmul(
                            pavs[i],
                            lhsT=v_sb[:, j, 2 * m + i, :],
                            rhs=ex[:, i, :],
                            start=(j == 0),
                            stop=(j == nkt - 1),
                        )
                    drain_piece()
                # PSUM->SBUF copies stay off the scalar engine: scalar is
                # saturated by the softmax exp stream (the phase bottleneck)
                for i in range(2):
                    nc.vector.tensor_copy(
                        out=den_sb[:, i, :],
                        in_=pavs[i][DH : DH + 1, :],
                    )
                    nc.vector.tensor_copy(
                        out=aT_sb[64 * i : 64 * i + 64, m,
                                  q0 : q0 + QT],
                        in_=pavs[i][0:DH, :],
                    )
                _recip_fast(
                    nc,
                    rec_sb[:, 2 * m * QT : 2 * m * QT + 2 * QT],
                    den_sb.rearrange("p a t -> p (a t)"),
                )
            pieces = []
            sth = {}

            def norm(m, q0=q0, rec_sb=rec_sb):
                # normalize by softmax denominator (paired PE broadcast)
                rbt = pscp.tile([128, QT], FP, tag="cp", name="rbt")
                for i in range(2):
                    h = 2 * m + i
                    nc.tensor.matmul(
                        rbt[64 * i : 64 * i + 64, :],
                        lhsT=ones_row[:, 0:64],
                        rhs=rec_sb[:, h * QT : h * QT + QT],
                        start=True,
                        stop=True,
                        tile_position=(0, 64 * i),
                    )
                nc.vector.tensor_mul(
                    aT_sb[:, m, q0 : q0 + QT],
                    aT_sb[:, m, q0 : q0 + QT],
                    rbt,
                )

            def cproj(dc, q0=q0, sth=sth):
                # partial c_proj for this q tile (bf16 payload halves the
                # bounce DMA and the ReduceScatter wire)
                if dc == 0:
                    sth["st"] = attR.tile(
                        [128, KC, QT], BF, tag="prst", name="prst"
                    )
                pt = pscp.tile([128, QT], FP, tag="cp", name="pt")
                for kc in range(QC):
                    nc.tensor.matmul(
                        pt,
                        lhsT=wpr_sb[:, kc, 128 * dc : 128 * dc + 128],
                        rhs=aT_sb[:, kc, q0 : q0 + QT],
                        start=(kc == 0),
                        stop=(kc == QC - 1),
                    )
                # vector, not scalar: scalar's queue carries the exp
                # stream and a copy here would stall the next softmax
                nc.vector.tensor_copy(out=st_slice(sth, dc), in_=pt)

            def st_slice(sth, dc):
                return sth["st"][:, dc, :]

            def fin(g=g, gi=gi, sth=sth):
                ch, th = g % 2, g // 2
                nc.sync.dma_start(a_bounce[ch][th], sth["st"])
                if gi in (1, 3):
                    # both halves of chunk ch are now written -> exchange,
                    # then pull the result into SBUF immediately (gpsimd
                    # queue: stalls only this queue until the RS lands)
                    nc.gpsimd.collective_compute(
                        "ReduceScatter",
                        OP.add,
                        replica_groups=[[0, 1], [2, 3], [4, 5], [6, 7]],
                        ins=[a_bounce[ch].opt()],
                        outs=[rs_out[ch].opt()],
                    )
                    nc.gpsimd.dma_start(rs_sbs[ch], rs_out[ch])

            for m in range(QC):
                pieces.append(lambda m=m: norm(m))
            for dc in range(KC):
                pieces.append(lambda dc=dc: cproj(dc))
            pieces.append(fin)

            if gi == 3:
                for p in pieces:
                    p()
            else:
                tail_q.extend(pieces)
        while tail_q:
            tail_q.pop(0)()


def _emit_ffn(tc, outT, nT_sb, n8_sb, xh_sb, rs_sbs, wfc_sb, wfc2_sb, bpr_sb,
              gb1_sb, gb2_sb, bfc_sb, bfc2_sb, ones128, eps_sb, ones_row):
    nc = tc.nc
    with (
        tc.tile_pool(name="ln", bufs=2) as ln,
        tc.tile_pool(name="lnb", bufs=2) as lnb,
        tc.tile_pool(name="lnsq", bufs=2) as lnsq,
        tc.tile_pool(name="ffn", bufs=1) as ffn,
        tc.tile_pool(name="ffy", bufs=1) as ffy,
        tc.tile_pool(name="pls", bufs=2, space="PSUM") as pls,
        tc.tile_pool(name="pub", bufs=1, space="PSUM") as pub,
        tc.tile_pool(name="psfc", bufs=2, space="PSUM") as psfc,
        tc.tile_pool(name="psf2", bufs=2, space="PSUM") as psf2,
    ):
        hT_sbs = {}

        def ln1(ht):
            t0 = ht * QT
            rs_sb = rs_sbs[ht]
            y1_sb = ln.tile([128, KC, QT], BF, tag="y1", name="y1")
            for c in range(KC):
                nc.vector.scalar_tensor_tensor(
                    out=y1_sb[:, c, :],
                    in0=rs_sb[:, c, :],
                    scalar=bpr_sb[:, c : c + 1],
                    in1=xh_sb[:, c, t0 : t0 + QT],
                    op0=OP.add,
                    op1=OP.add,
                )
            # LN1 output is also needed as fp8 (the fc1 DoubleRow rhs),
            # DC-shifted by gb1 so the fp8 grid covers the +-1-ish
            # normalized part (the gb1@w_fc term is folded into fc1's bias
            # host-side). Scalar is idle here (the exp stream is done).
            _emit_ln(
                tc, nT_sb[:, :, t0 : t0 + QT], y1_sb, gb1_sb, ones128, eps_sb,
                ones_row, lnb, lnsq, pls, pub,
                out_dma=lambda c: nc.vector.tensor_scalar_sub(
                    n8_sb[:, c, t0 : t0 + QT],
                    nT_sb[:, c, t0 : t0 + QT],
                    gb1_sb[:, c : c + 1],
                ),
            )

        def fc(ht):
            t0 = ht * QT
            hT_sb = ffn.tile([128, FC, QT], BF, tag="hT", name="hT")
            hT_sbs[ht] = hT_sb
            for fci in range(FC):
                ps = psfc.tile([128, QT], FP)
                for kp in range(KC // 2):
                    nc.tensor.matmul(
                        ps,
                        lhsT=wfc_sb[:, 2 * kp : 2 * kp + 2,
                                    128 * fci : 128 * fci + 128],
                        rhs=n8_sb[:, 2 * kp : 2 * kp + 2, t0 : t0 + QT],
                        start=(kp == 0),
                        stop=(kp == KC // 2 - 1),
                        perf_mode=DR,
                    )
                # ps = FS*((n-gb1)@w_fc); bfc carries b_fc + gb1@w_fc
                nc.scalar.activation(
                    out=hT_sb[:, fci, :],
                    in_=ps,
                    func=AF.Relu,
                    bias=bfc_sb[:, fci : fci + 1],
                    scale=1.0 / FS,
                )

        def fc2_ln2(ht, mid_cb=None):
            t0 = ht * QT
            nT_c = nT_sb[:, :, t0 : t0 + QT]
            hT_sb = hT_sbs.pop(ht)
            y_sb = ffy.tile([128, KC, QT], BF, tag="y", name="y")
            # LN2 stats accumulate inline as each y chunk lands, so only
            # the short serial chain remains after the last fc2 group
            # (pls bufs=2: ln1(1) is injected mid-loop and must not share
            # this accumulator's bank)
            lps = pls.tile([64, QT], FP, tag="s12", name="lps")
            for dc in range(KC):
                if dc == 3 and mid_cb is not None:
                    # injected here so its serial LN chain overlaps the
                    # rest of this chunk's fc2 matmul stream
                    mid_cb()
                ps2 = psf2.tile([128, QT], FP)
                for k in range(FC):
                    nc.tensor.matmul(
                        ps2,
                        lhsT=wfc2_sb[:, k, 128 * dc : 128 * dc + 128],
                        rhs=hT_sb[:, k, :],
                        start=(k == 0),
                        stop=(k == FC - 1),
                    )
                nc.vector.scalar_tensor_tensor(
                    out=y_sb[:, dc, :],
                    in0=ps2,
                    scalar=bfc2_sb[:, dc : dc + 1],
                    in1=nT_c[:, dc, :],
                    op0=OP.add,
                    op1=OP.add,
                )
                sq = lnsq.tile([128, QT], BF, tag="sq")
                nc.vector.tensor_mul(sq, y_sb[:, dc, :], y_sb[:, dc, :])
                nc.tensor.matmul(
                    lps[0:1, :], lhsT=ones128, rhs=y_sb[:, dc, :],
                    start=(dc == 0), stop=(dc == KC - 1),
                    tile_position=(0, 0),
                )
                nc.tensor.matmul(
                    lps[32:33, :], lhsT=ones128, rhs=sq,
                    start=(dc == 0), stop=(dc == KC - 1),
                    tile_position=(0, 32),
                )
            o_sb = ffy.tile([128, KC, QT], FP, tag="o", name="o")
            engs = [nc.sync, nc.scalar]
            _emit_ln(
                tc, o_sb, y_sb, gb2_sb, ones128, eps_sb, ones_row,
                lnb, lnsq, pls, pub,
                out_dma=lambda c: engs[c % 2].dma_start(
                    outT[:, c, t0 : t0 + QT], o_sb[:, c, :]
                ),
                stats_ps=lps,
            )

        # fc2_ln2(0) is emitted BEFORE ln1(1): ln1(1) blocks on the second
        # ReduceScatter, and the strict-FIFO engine queues would stall all
        # of chunk-0's remaining work behind that wait
        ln1(0)
        fc(0)
        fc2_ln2(0, mid_cb=lambda: ln1(1))
        fc(1)
        fc2_ln2(1)


def _emit_ln(tc, out_sb, y_sb, gb_sb, ones128, eps_sb, ones_row, bpool,
             sqpool, pspool, bcpool, out_dma=None, stats_ps=None):
    """out = gb + (y - mean(y)) * rsqrt(var(y) + eps), feature-major bf16.

    y_sb: [128, KC, QT]. mean/var run over the full feature dim
    (partitions x KC chunks) via column-tiled ones-matmuls on the PE
    (sum at partition 0, sum-of-squares at partition 32 -> concurrent).
    The apply is fused as out = (y*RS + gb) - C with C = broadcast(u*rstd).
    """
    nc = tc.nc
    w = QT
    if stats_ps is not None:
        # caller already accumulated sum / sum-of-squares (inline with
        # its matmul stream); only the chain + apply remain here
        ps = stats_ps
    else:
        ps = pspool.tile([64, w], FP, tag="s12")
        for c in range(KC):
            sq = sqpool.tile([128, w], BF, tag="sq")
            nc.vector.tensor_mul(sq, y_sb[:, c, :], y_sb[:, c, :])
            nc.tensor.matmul(
                ps[0:1, :],
                lhsT=ones128,
                rhs=y_sb[:, c, :],
                start=(c == 0),
                stop=(c == KC - 1),
                tile_position=(0, 0),
            )
            nc.tensor.matmul(
                ps[32:33, :],
                lhsT=ones128,
                rhs=sq,
                start=(c == 0),
                stop=(c == KC - 1),
                tile_position=(0, 32),
            )
    u = bpool.tile([1, w], FP, tag="u")
    m2 = bpool.tile([1, w], FP, tag="m2")
    nc.scalar.activation(out=u, in_=ps[0:1, :], func=AF.Copy, scale=1.0 / D)
    nc.scalar.activation(out=m2, in_=ps[32:33, :], func=AF.Copy, scale=1.0 / D)
    var = bpool.tile([1, w], FP, tag="var")
    nc.vector.tensor_mul(var, u, u)
    nc.vector.tensor_tensor(out=var, in0=m2, in1=var, op=OP.subtract)
    sd = bpool.tile([1, w], FP, tag="sd")
    nc.scalar.activation(out=sd, in_=var, func=AF.Sqrt, bias=eps_sb)
    rstd = bpool.tile([1, w], BF, tag="rstd")
    _recip_fast(nc, rstd, sd)
    crow = bpool.tile([1, w], BF, tag="crow")
    nc.vector.tensor_mul(crow, u, rstd)
    RS = bcpool.tile([128, w], FP, tag="RS")
    CB = bcpool.tile([128, w], FP, tag="CB")
    nc.tensor.matmul(RS, lhsT=ones_row, rhs=rstd, start=True, stop=True)
    nc.tensor.matmul(CB, lhsT=ones_row, rhs=crow, start=True, stop=True)
    for c in range(KC):
        t = sqpool.tile([128, w], BF, tag="t")
        nc.vector.tensor_mul(t, y_sb[:, c, :], RS)
        nc.vector.scalar_tensor_tensor(
            out=out_sb[:, c, :],
            in0=t,
            scalar=gb_sb[:, c : c + 1],
            in1=CB,
            op0=OP.add,
            op1=OP.subtract,
        )
        if out_dma is not None:
            out_dma(c)


# ------------------------------------------------------------------ host side

def _pm(a2d):
    """[D_any, N] -> partition-major [128, D_any//128, N] (f = c*128 + p)."""
    d, n = a2d.shape
    return np.ascontiguousarray(a2d.reshape(d // 128, 128, n).transpose(1, 0, 2))


def _pmb(vec):
    """[D_any] -> [128, D_any//128] fp32 bias layout."""
    return np.ascontiguousarray(vec.reshape(-1, 128).T).astype(np.float32)


def shard_inputs(inputs):
    """Full inputs -> 8 per-core input dicts (numpy, partition-major bf16)."""
    bf = ml_dtypes.bfloat16
    f8 = ml_dtypes.float8_e4m3
    FS_NP = np.float32(FS)
    x = np.asarray(inputs["x"], np.float32)                        # [B, S, D]
    w_attn = np.asarray(inputs["w_attn"], np.float32)              # [D, 3D]
    b_attn = np.asarray(inputs["b_attn"], np.float32)
    w_proj = np.asarray(inputs["w_proj"], np.float32)
    b_proj = np.asarray(inputs["b_proj"], np.float32)
    w_fc = np.asarray(inputs["w_fc"], np.float32)
    b_fc = np.asarray(inputs["b_fc"], np.float32)
    w_fc2 = np.asarray(inputs["w_fc2"], np.float32)
    b_fc2 = np.asarray(inputs["b_fc2"], np.float32)
    gb1 = (np.asarray(inputs["ln1_g"]) + np.asarray(inputs["ln1_b"])).astype(
        np.float32
    )
    gb2 = (np.asarray(inputs["ln2_g"]) + np.asarray(inputs["ln2_b"])).astype(
        np.float32
    )
    # mask[p, r, t] = 1.0 if (128r + p) <= t else 0.0
    ki = (np.arange(DQT * 128).reshape(DQT, 128)).T[:, :, None]    # [128, DQT, 1]
    mask = (ki <= np.arange(QT)[None, None, :]).astype(bf)

    in_maps = []
    for c in range(N_CORES):
        b, p = c // 2, c % 2
        f0 = p * FH                      # first owned q/k/v feature
        xT = x[b].T                                              # [D, S]
        xhT = x[b, p * HALF : (p + 1) * HALF].T                  # [D, HALF]
        wqk = np.concatenate(
            [w_attn[:, f0 : f0 + FH], w_attn[:, D + f0 : D + f0 + FH]], axis=1
        )
        bqk = np.concatenate([b_attn[f0 : f0 + FH], b_attn[D + f0 : D + f0 + FH]])
        wv_ = w_attn[:, 2 * D + f0 : 2 * D + f0 + FH]
        bv_ = b_attn[2 * D + f0 : 2 * D + f0 + FH]
        wpr_ = w_proj[f0 : f0 + FH, :]
        in_maps.append(
            dict(
                xpm=_pm(xT).astype(bf),
                xh=_pm(xhT).astype(bf),
                wqk=_pm(wqk).astype(bf),
                bqk=_pmb(bqk),
                wv=_pm(wv_).astype(bf),
                bv=np.broadcast_to(
                    bv_.reshape(HL, DH).astype(bf), (128, HL, DH)
                ).copy(),
                wpr=_pm(wpr_).astype(bf),
                bpr=_pmb(b_proj),
                gb1=_pmb(gb1),
                gb2=_pmb(gb2),
                # fc1 weights: x16 into fp8-e4m3 (clip at TRN's +-240 max);
                # its rhs is n - gb1, so fold gb1@w_fc into the bias
                wfc=_pm(np.clip(w_fc * FS_NP, -240, 240)).astype(f8),
                bfc=_pmb(b_fc + gb1 @ w_fc),
                wfc2=_pm(w_fc2).astype(bf),
                bfc2=_pmb(b_fc2),
                mask=np.ascontiguousarray(mask),
            )
        )
    return in_maps


_IN_SPECS = dict(
    xpm=((128, KC, S), BF),
    xh=((128, KC, HALF), BF),
    wqk=((128, KC, 2 * FH), BF),
    bqk=((128, 2 * QC), FP),
    wv=((128, KC, FH), BF),
    bv=((128, HL, DH), BF),
    wpr=((128, QC, D), BF),
    bpr=((128, KC), FP),
    gb1=((128, KC), FP),
    gb2=((128, KC), FP),
    wfc=((128, KC, F), F8),
    bfc=((128, FC), FP),
    wfc2=((128, FC, D), BF),
    bfc2=((128, KC), FP),
    mask=((128, DQT, QT), BF),
)


def build_module():
    nc = bacc.Bacc(
        "TRN2", target_bir_lowering=False, debug=False, num_devices=N_CORES
    )
    ins = {
        name: nc.dram_tensor(name, list(shape), dt, kind="ExternalInput").ap()
        for name, (shape, dt) in _IN_SPECS.items()
    }
    outs = {
        "outT": nc.dram_tensor(
            "outT", [128, KC, HALF], FP, kind="ExternalOutput"
        ).ap(),
    }
    with tile.TileContext(nc) as tc:
        emit_block(tc, outs, ins)
    nc.compile()
    return nc


def unshard_output(results):
    """Per-core outT [128, KC, HALF] fp32 -> full [B, S, D]."""
    y = np.empty((B, S, D), np.float32)
    for c in range(N_CORES):
        b, p = c // 2, c % 2
        o = np.asarray(results[c]["outT"], np.float32)   # [128, KC, HALF]
        feat_major = o.transpose(1, 0, 2).reshape(D, HALF)
        y[b, p * HALF : (p + 1) * HALF, :] = feat_major.T
    return y


def kernel(**inputs):
    nc = build_module()
    in_maps = shard_inputs(inputs)
    res = run_bass_kernel_spmd(nc, in_maps, core_ids=list(range(N_CORES)))
    return unshard_output(res.results)



# revision 99
# speedup vs baseline: 1.0550x; 1.0550x over previous
"""Trainium2 Bass kernel for a GPT-2 style transformer block (B=4, S=2048, D=768).

Sharding (8 NeuronCores, one SPMD program):
  core c = (b, p): b = c // 2 (batch), p = c % 2 (tensor-parallel rank).
  - Attention is head-split: p=0 owns heads 0..5, p=1 owns heads 6..11,
    each over the FULL sequence of its batch.
  - c_attn / c_proj are computed only for the owned heads; the partial
    c_proj outputs are summed + token-scattered with a ReduceScatter over
    core pairs [[0,1],[2,3],[4,5],[6,7]] (bf16 payload).
  - LN1 / FFN / LN2 are token-split: p owns tokens [p*1024, (p+1)*1024).

All matmuls run in bf16 (fp32 PSUM accumulation). All DRAM tensors are
pre-shuffled host-side into partition-major [128, ...] layouts so DMAs are
contiguous per partition. Scores matmuls pack 2 heads via PE row tiling
(K=64 each); LN mean/var matmuls pack via PE column tiling. Softmax
denominators come free from a ones-column appended to V (M=65 AV matmuls);
reciprocals use the fast custom-DVE approximation.
"""

import numpy as np
import ml_dtypes

import concourse.bass as bass
import concourse.mybir as mybir
import concourse.tile as tile
from concourse import bacc
from concourse.bass_utils import run_bass_kernel_spmd

# ---------------------------------------------------------------- constants
B = 4
S = 2048
D = 768
H = 12
DH = 64
F = 3072
EPS = 1e-5

N_CORES = 8
HL = H // 2            # heads per core (6)
FH = HL * DH           # per-core attention feature width (384)
KC = D // 128          # contraction chunks over D (6)
QC = FH // 128         # feature chunks for per-core q or k (3)
FC = F // 128          # fc feature chunks (24)
QT = 512               # attention q-tile width
GQ = S // QT           # q tiles over full sequence (4)
DQT = QT // 128        # k-blocks per q tile width (4)
NKT = S // 128         # k-blocks over full sequence (16)
HALF = S // 2          # tokens owned per core for FFN/LN (1024)
TT = HALF // QT        # 512-token tiles per half (2)

FP = mybir.dt.float32
FPR = mybir.dt.float32r
BF = mybir.dt.bfloat16
F8 = mybir.dt.float8e4
DR = mybir.MatmulPerfMode.DoubleRow
FS = 16.0                # host-side scale on fp8 fc weights (dynamic range)

AF = mybir.ActivationFunctionType
OP = mybir.AluOpType


def _recip_fast(nc, out, in_):
    """out = approx 1/in_ (custom DVE op); out dtype may be bf16 (rounds
    on writeback), in_ must be fp32/PSUM-fp32 (bit-trick seed needs fp32
    layout)."""
    from concourse.dve_ops import RECIP_APPROX_FAST_CONSTS, RECIPROCAL_APPROX_FAST

    c = RECIP_APPROX_FAST_CONSTS
    return nc.vector._custom_dve(
        RECIPROCAL_APPROX_FAST, out=out, in0=in_, s0=c["s0"], s1=c["s1"],
        imm2=c["imm2"],
    )


def emit_block(tc, outs, ins):
    nc = tc.nc
    with nc.allow_low_precision(reason="bf16 matmul pipeline by design"):
        _emit_block_inner(tc, outs, ins)


def _emit_block_inner(tc, outs, ins):
    nc = tc.nc
    outT = outs["outT"]

    xpm = ins["xpm"]             # [128, KC, S] bf16
    xh = ins["xh"]               # [128, KC, HALF] bf16
    wqk = ins["wqk"]             # [128, KC, 2*FH] bf16
    bqk = ins["bqk"]             # [128, 2*QC] fp32
    wv = ins["wv"]               # [128, KC, FH] bf16
    bv = ins["bv"]               # [128, HL, DH] bf16 (broadcast materialized)
    wpr = ins["wpr"]             # [128, QC, D] bf16
    bpr = ins["bpr"]             # [128, KC] fp32
    gb1 = ins["gb1"]             # [128, KC] fp32
    gb2 = ins["gb2"]             # [128, KC] fp32
    wfc = ins["wfc"]             # [128, KC, F] bf16
    bfc = ins["bfc"]             # [128, FC] fp32
    wfc2 = ins["wfc2"]           # [128, FC, D] bf16
    bfc2 = ins["bfc2"]           # [128, KC] fp32
    mask = ins["mask"]           # [128, DQT, QT] bf16: mask[p, r, t] = (128r+p) <= t

    with (
        tc.tile_pool(name="const", bufs=1) as const,
        tc.tile_pool(name="dram", bufs=1, space="DRAM") as dram,
    ):
        # ---------------- constants
        # phase-1 consts go on the scalar HWDGE queue (cheap fixed cost);
        # FFN consts are deferred to ffn_loads() -- putting them on the
        # gpsimd SWDGE queue up front costs ~2us fixed EACH and delays the
        # attention weight loads behind them
        mask_sb = const.tile([128, DQT, QT], BF)
        bqk_sb = const.tile([128, 2 * QC], FP)
        nc.scalar.dma_start(bqk_sb, bqk)
        bv_sb = const.tile([128, HL, DH], BF)
        nc.scalar.dma_start(bv_sb, bv)
        bpr_sb = const.tile([128, KC], FP)
        gb1_sb = const.tile([128, KC], FP)
        gb2_sb = const.tile([128, KC], FP)
        bfc_sb = const.tile([128, FC], FP)
        bfc2_sb = const.tile([128, KC], FP)
        ones128 = const.tile([128, 1], BF)
        nc.vector.memset(ones128, 1.0)
        ones_row = const.tile([1, 128], BF)
        nc.vector.memset(ones_row, 1.0)
        eps_sb = const.tile([1, 1], FP)
        nc.vector.memset(eps_sb, EPS)
        # mask load is issued later (gpsimd queue, behind the attention
        # weights) so phase-1's x/wqk loads aren't delayed behind it

        # one DRAM tile per chunk so chunk-0 consumers don't falsely
        # depend on chunk-1's collective (dep tracking is per-tile)
        a_bounce = [
            dram.tile([2, 128, KC, QT], BF, tag=f"ab{c}", name=f"ab{c}")
            for c in range(TT)
        ]
        rs_out = [
            dram.tile([128, KC, QT], BF, tag=f"rs{c}", name=f"rs{c}")
            for c in range(TT)
        ]

        with tc.tile_pool(name="ffw", bufs=1) as ffw:
            # fc1 weights fp8 (x16 host-scaled, DoubleRow contracts 256
            # features per MM); fc2 stays bf16 -- quantizing both matmuls
            # pushes the output past the accuracy budget
            wfc_sb = ffw.tile([128, KC, F], F8)
            wfc2_sb = ffw.tile([128, FC, D], BF)

            with tc.tile_pool(name="nt", bufs=1) as npool:
                nT_sb = npool.tile([128, KC, HALF], BF)
                n8_sb = npool.tile([128, KC, HALF], F8)
                xh_sb = npool.tile([128, KC, HALF], BF)
                rs_sbs = [
                    npool.tile([128, KC, QT], BF, tag=f"rsb{i}", name=f"rsb{i}")
                    for i in range(TT)
                ]
                def ffn_loads():
                    nc.scalar.dma_start(bpr_sb, ins["bpr"])
                    nc.scalar.dma_start(gb1_sb, ins["gb1"])
                    nc.scalar.dma_start(gb2_sb, ins["gb2"])
                    nc.scalar.dma_start(bfc_sb, ins["bfc"])
                    nc.scalar.dma_start(bfc2_sb, ins["bfc2"])
                    nc.scalar.dma_start(xh_sb, xh)
                    nc.scalar.dma_start(wfc_sb, ins["wfc"])
                    nc.scalar.dma_start(wfc2_sb, ins["wfc2"])

                _emit_qkv_attention(
                    tc, ins, a_bounce, rs_out, rs_sbs, mask_sb, bqk_sb,
                    bv_sb, ones_row, ffn_loads,
                )

                # ====== phases 4+5: LN1 -> FFN -> LN2 per 512-token chunk ==
                _emit_ffn(
                    tc, outT, nT_sb, n8_sb, xh_sb, rs_sbs, wfc_sb, wfc2_sb,
                    bpr_sb, gb1_sb, gb2_sb, bfc_sb, bfc2_sb, ones128, eps_sb,
                    ones_row,
                )


def _emit_qkv_attention(tc, ins, a_bounce, rs_out, rs_sbs, mask_sb, bqk_sb,
                        bv_sb, ones_row, ffn_loads):
    nc = tc.nc
    xpm = ins["xpm"]
    wqk = ins["wqk"]
    wv = ins["wv"]
    wpr = ins["wpr"]

    with tc.tile_pool(name="kqv", bufs=1) as kqv:
        # persistent attention activations (bf16)
        kT_sb = kqv.tile([128, QC, S], BF)            # k, feature-major
        qT_sb = kqv.tile([128, QC, S], BF)            # q, feature-major
        v_sb = kqv.tile([128, NKT, HL, DH + 1], BF)   # v token-major + ones
        nc.vector.memset(v_sb[:, :, :, DH : DH + 1], 1.0)

        # ================ phase 1: qkv projections =====================
        with (
            tc.tile_pool(name="p1", bufs=2) as p1,
            tc.tile_pool(name="p1w", bufs=1) as p1w,
            tc.tile_pool(name="psqk", bufs=4, space="PSUM") as psqk,
            tc.tile_pool(name="psv", bufs=2, space="PSUM") as psv,
        ):
            # all phase-1 loads issued up front as few, large DMAs (HWDGE
            # fixed cost is ~0.6us EACH): wqk split across the two HWDGE
            # queues, x split 3 ways (sync/scalar/gpsimd) so the first-MM
            # gate isn't one queue's serial drain; mask goes LAST on
            # gpsimd (first needed only when attention starts). FFN
            # weights only start after these (ffn_loads).
            KH = KC // 2
            wqk_sb = p1w.tile([128, KC, 2 * FH], BF)
            nc.sync.dma_start(wqk_sb[:, 0:KH, :], wqk[:, 0:KH, :])
            nc.scalar.dma_start(wqk_sb[:, KH:, :], wqk[:, KH:, :])
            wv_sb = p1w.tile([128, KC, FH], BF)
            nc.gpsimd.dma_start(wv_sb, wv)
            x_ts = []
            for half in range(2):
                t0 = half * HALF
                x_t = p1.tile([128, KC, HALF], BF, tag="xT")
                x_ts.append(x_t)
                nc.sync.dma_start(
                    x_t[:, 0:2, :], xpm[:, 0:2, t0 : t0 + HALF]
                )
                nc.scalar.dma_start(
                    x_t[:, 2:4, :], xpm[:, 2:4, t0 : t0 + HALF]
                )
                nc.gpsimd.dma_start(
                    x_t[:, 4:6, :], xpm[:, 4:6, t0 : t0 + HALF]
                )
            nc.gpsimd.dma_start(mask_sb, ins["mask"])

            for half in range(2):
                t0 = half * HALF
                x_t = x_ts[half]
                # q / k feature-major: out[feat_chunk, tokens]
                for fc in range(2 * QC):
                    for ttt in range(TT):
                        ps = psqk.tile([128, QT], FP)
                        for k in range(KC):
                            nc.tensor.matmul(
                                ps,
                                lhsT=wqk_sb[:, k, 128 * fc : 128 * fc + 128],
                                rhs=x_t[:, k, QT * ttt : QT * ttt + QT],
                                start=(k == 0),
                                stop=(k == KC - 1),
                            )
                        dst = qT_sb if fc < QC else kT_sb
                        cc = fc if fc < QC else fc - QC
                        nc.vector.tensor_scalar_add(
                            dst[:, cc, t0 + QT * ttt : t0 + QT * ttt + QT],
                            ps,
                            bqk_sb[:, fc : fc + 1],
                        )
                # v token-major: out[token_chunk, v features]
                for tcc in range(HALF // 128):
                    ps = psv.tile([128, FH], FP)
                    for k in range(KC):
                        nc.tensor.matmul(
                            ps,
                            lhsT=x_t[:, k, 128 * tcc : 128 * tcc + 128],
                            rhs=wv_sb[:, k, :],
                            start=(k == 0),
                            stop=(k == KC - 1),
                        )
                    tok = half * (HALF // 128) + tcc
                    nc.vector.tensor_add(
                        v_sb[:, tok, :, 0:DH],
                        ps.rearrange("p (h d) -> p h d", h=HL),
                        bv_sb,
                    )

        # FFN weights + residual stream: issued only now so phase-1's
        # x/wqk loads aren't stuck behind 9.4MB of FFN weights
        ffn_loads()
        # ============ phase 2+3: attention, normalize, c_proj ==========
        _emit_attention(
            tc, a_bounce, rs_out, rs_sbs, kT_sb, qT_sb, v_sb, wpr, mask_sb,
            ones_row,
        )


def _emit_attention(tc, a_bounce, rs_out, rs_sbs, kT_sb, qT_sb, v_sb, wpr,
                    mask_sb, ones_row):
    nc = tc.nc
    with (
        tc.tile_pool(name="att", bufs=1) as att,
        tc.tile_pool(name="attR", bufs=2) as attR,
        tc.tile_pool(name="pss", bufs=2, space="PSUM") as pss,
        tc.tile_pool(name="psav", bufs=1, space="PSUM") as psav,
        tc.tile_pool(name="pscp", bufs=2, space="PSUM") as pscp,
    ):
        aT_sb = att.tile([128, QC, S], BF)    # attention out, feature-major
        wpr_sb = att.tile([128, QC, D], BF)
        nc.sync.dma_start(wpr_sb, wpr)
        # manual exp ring: masked-out columns of diagonal blocks are
        # never re-exp'd; the mask multiply zeroes them against stale
        # (finite) data, seeded by this one-time memset.
        # 4 ring slots: with 3, q-tiles whose block count is 1 mod 3
        # serialize the next head-pair's first exp behind this pair's last
        # AV via the slot-0 write-after-read
        exm = att.tile([128, 4, 2, QT], BF)
        nc.vector.memset(exm, 0.0)

        # the per-g tail (normalize + c_proj + ReduceScatter) is split into
        # ~10 small pieces drained ONE PER j-ITERATION of the next g's
        # attention loops: each piece's PE/DVE work hides in the slack
        # under the exp stream instead of stalling softmax ~14us per g
        tail_q = []

        def drain_piece():
            if tail_q:
                tail_q.pop(0)()

        for gi, g in enumerate([0, 2, 1, 3]):
            q0 = g * QT
            nkt = DQT * (g + 1)
            rec_sb = attR.tile([1, HL * QT], BF, tag="rec")
            for m in range(QC):  # head pairs (2m, 2m+1)
                den_sb = attR.tile([1, 2, QT], FP, tag="den")
                pavs = [
                    psav.tile(
                        [DH + 1, QT], FP, tag=f"pav{i}", name=f"pav{i}"
                    )
                    for i in range(2)
                ]
                def scores(j):
                    ps = pss.tile([128, 2, QT], FP, tag="ss", name="ps")
                    jd = j - DQT * g
                    c0 = 128 * jd if jd > 0 else 0
                    # 2 heads row-tiled (K=64 each) -> concurrent MMs;
                    # diagonal blocks stream only the causally-live cols
                    for i in range(2):
                        hr = i * 64
                        nc.tensor.matmul(
                            ps[:, i, c0:],
                            lhsT=kT_sb[
                                hr : hr + 64, m, 128 * j : 128 * j + 128
                            ],
                            rhs=qT_sb[hr : hr + 64, m, q0 + c0 : q0 + QT],
                            start=True,
                            stop=True,
                        )
                    return ps

                ps = scores(0)
                for j in range(nkt):
                    jd = j - DQT * g
                    c0 = 128 * jd if jd > 0 else 0
                    ex = exm[:, j % 4, :, :]
                    nc.scalar.activation(
                        out=ex[:, :, c0:],
                        in_=ps[:, :, c0:],
                        func=AF.Exp,
                    )
                    # software pipeline: scores_{j+1} is emitted BEFORE this
                    # j's AV matmuls. The PE queue is strict FIFO and AV_j
                    # waits on exp_j -- with scores_{j+1} behind AV_j the
                    # loop serializes at (exp+AV+scores) per step instead of
                    # running exp-bound with the PE work hidden under it.
                    if j + 1 < nkt:
                        ps = scores(j + 1)
                    if jd >= 0:
                        for i in range(2):
                            nc.vector.tensor_mul(
                                ex[:, i, :], ex[:, i, :], mask_sb[:, jd, :]
                            )
                    for i in range(2):
                        nc.tensor.matmul(
                            pavs[i],
                            lhsT=v_sb[:, j, 2 * m + i, :],
                            rhs=ex[:, i, :],
                            start=(j == 0),
                            stop=(j == nkt - 1),
                        )
                    drain_piece()
                # PSUM->SBUF copies stay off the scalar engine: scalar is
                # saturated by the softmax exp stream (the phase bottleneck)
                for i in range(2):
                    nc.vector.tensor_copy(
                        out=den_sb[:, i, :],
                        in_=pavs[i][DH : DH + 1, :],
                    )
                    nc.vector.tensor_copy(
                        out=aT_sb[64 * i : 64 * i + 64, m,
                                  q0 : q0 + QT],
                        in_=pavs[i][0:DH, :],
                    )
                _recip_fast(
                    nc,
                    rec_sb[:, 2 * m * QT : 2 * m * QT + 2 * QT],
                    den_sb.rearrange("p a t -> p (a t)"),
                )
            pieces = []
            sth = {}

            def norm(m, q0=q0, rec_sb=rec_sb):
                # normalize by softmax denominator (paired PE broadcast)
                rbt = pscp.tile([128, QT], FP, tag="cp", name="rbt")
                for i in range(2):
                    h = 2 * m + i
                    nc.tensor.matmul(
                        rbt[64 * i : 64 * i + 64, :],
                        lhsT=ones_row[:, 0:64],
                        rhs=rec_sb[:, h * QT : h * QT + QT],
                        start=True,
                        stop=True,
                        tile_position=(0, 64 * i),
                    )
                nc.vector.tensor_mul(
                    aT_sb[:, m, q0 : q0 + QT],
                    aT_sb[:, m, q0 : q0 + QT],
                    rbt,
                )

            def cproj(dc, q0=q0, sth=sth):
                # partial c_proj for this q tile (bf16 payload halves the
                # bounce DMA and the ReduceScatter wire)
                if dc == 0:
                    sth["st"] = attR.tile(
                        [128, KC, QT], BF, tag="prst", name="prst"
                    )
                pt = pscp.tile([128, QT], FP, tag="cp", name="pt")
                for kc in range(QC):
                    nc.tensor.matmul(
                        pt,
                        lhsT=wpr_sb[:, kc, 128 * dc : 128 * dc + 128],
                        rhs=aT_sb[:, kc, q0 : q0 + QT],
                        start=(kc == 0),
                        stop=(kc == QC - 1),
                    )
                # vector, not scalar: scalar's queue carries the exp
                # stream and a copy here would stall the next softmax
                nc.vector.tensor_copy(out=st_slice(sth, dc), in_=pt)

            def st_slice(sth, dc):
                return sth["st"][:, dc, :]

            def fin(g=g, gi=gi, sth=sth):
                ch, th = g % 2, g // 2
                nc.sync.dma_start(a_bounce[ch][th], sth["st"])
                if gi in (1, 3):
                    # both halves of chunk ch are now written -> exchange,
                    # then pull the result into SBUF immediately (gpsimd
                    # queue: stalls only this queue until the RS lands)
                    nc.gpsimd.collective_compute(
                        "ReduceScatter",
                        OP.add,
                        replica_groups=[[0, 1], [2, 3], [4, 5], [6, 7]],
                        ins=[a_bounce[ch].opt()],
                        outs=[rs_out[ch].opt()],
                    )
                    nc.gpsimd.dma_start(rs_sbs[ch], rs_out[ch])

            for m in range(QC):
                pieces.append(lambda m=m: norm(m))
            for dc in range(KC):
                pieces.append(lambda dc=dc: cproj(dc))
            pieces.append(fin)

            if gi == 3:
                for p in pieces:
                    p()
            else:
                tail_q.extend(pieces)
        while tail_q:
            tail_q.pop(0)()


def _emit_ffn(tc, outT, nT_sb, n8_sb, xh_sb, rs_sbs, wfc_sb, wfc2_sb, bpr_sb,
              gb1_sb, gb2_sb, bfc_sb, bfc2_sb, ones128, eps_sb, ones_row):
    nc = tc.nc
    with (
        tc.tile_pool(name="ln", bufs=2) as ln,
        tc.tile_pool(name="lnb", bufs=2) as lnb,
        tc.tile_pool(name="lnsq", bufs=2) as lnsq,
        tc.tile_pool(name="ffn", bufs=1) as ffn,
        tc.tile_pool(name="ffy", bufs=1) as ffy,
        tc.tile_pool(name="pls", bufs=2, space="PSUM") as pls,
        tc.tile_pool(name="pub", bufs=1, space="PSUM") as pub,
        tc.tile_pool(name="psfc", bufs=2, space="PSUM") as psfc,
        tc.tile_pool(name="psf2", bufs=2, space="PSUM") as psf2,
    ):
        hT_sbs = {}

        def ln1(ht):
            t0 = ht * QT
            rs_sb = rs_sbs[ht]
            y1_sb = ln.tile([128, KC, QT], BF, tag="y1", name="y1")
            for c in range(KC):
                nc.vector.scalar_tensor_tensor(
                    out=y1_sb[:, c, :],
                    in0=rs_sb[:, c, :],
                    scalar=bpr_sb[:, c : c + 1],
                    in1=xh_sb[:, c, t0 : t0 + QT],
                    op0=OP.add,
                    op1=OP.add,
                )
            # LN1 output is also needed as fp8 (the fc1 DoubleRow rhs),
            # DC-shifted by gb1 so the fp8 grid covers the +-1-ish
            # normalized part (the gb1@w_fc term is folded into fc1's bias
            # host-side). Scalar is idle here (the exp stream is done).
            _emit_ln(
                tc, nT_sb[:, :, t0 : t0 + QT], y1_sb, gb1_sb, ones128, eps_sb,
                ones_row, lnb, lnsq, pls, pub,
                out_dma=lambda c: nc.vector.tensor_scalar_sub(
                    n8_sb[:, c, t0 : t0 + QT],
                    nT_sb[:, c, t0 : t0 + QT],
                    gb1_sb[:, c : c + 1],
                ),
            )

        def fc(ht):
            t0 = ht * QT
            hT_sb = ffn.tile([128, FC, QT], BF, tag="hT", name="hT")
            hT_sbs[ht] = hT_sb
            for fci in range(FC):
                ps = psfc.tile([128, QT], FP)
                for kp in range(KC // 2):
                    nc.tensor.matmul(
                        ps,
                        lhsT=wfc_sb[:, 2 * kp : 2 * kp + 2,
                                    128 * fci : 128 * fci + 128],
                        rhs=n8_sb[:, 2 * kp : 2 * kp + 2, t0 : t0 + QT],
                        start=(kp == 0),
                        stop=(kp == KC // 2 - 1),
                        perf_mode=DR,
                    )
                # ps = FS*((n-gb1)@w_fc); bfc carries b_fc + gb1@w_fc
                nc.scalar.activation(
                    out=hT_sb[:, fci, :],
                    in_=ps,
                    func=AF.Relu,
                    bias=bfc_sb[:, fci : fci + 1],
                    scale=1.0 / FS,
                )

        def fc2_ln2(ht, mid_cb=None):
            t0 = ht * QT
            nT_c = nT_sb[:, :, t0 : t0 + QT]
            hT_sb = hT_sbs.pop(ht)
            y_sb = ffy.tile([128, KC, QT], BF, tag="y", name="y")
            # LN2 stats accumulate inline as each y chunk lands, so only
            # the short serial chain remains after the last fc2 group
            # (pls bufs=2: ln1(1) is injected mid-loop and must not share
            # this accumulator's bank)
            lps = pls.tile([64, QT], FP, tag="s12", name="lps")
            for dc in range(KC):
                if dc == 3 and mid_cb is not None:
                    # injected here so its serial LN chain overlaps the
                    # rest of this chunk's fc2 matmul stream
                    mid_cb()
                ps2 = psf2.tile([128, QT], FP)
                for k in range(FC):
                    nc.tensor.matmul(
                        ps2,
                        lhsT=wfc2_sb[:, k, 128 * dc : 128 * dc + 128],
                        rhs=hT_sb[:, k, :],
                        start=(k == 0),
                        stop=(k == FC - 1),
                    )
                nc.vector.scalar_tensor_tensor(
                    out=y_sb[:, dc, :],
                    in0=ps2,
                    scalar=bfc2_sb[:, dc : dc + 1],
                    in1=nT_c[:, dc, :],
                    op0=OP.add,
                    op1=OP.add,
                )
                sq = lnsq.tile([128, QT], BF, tag="sq")
                nc.vector.tensor_mul(sq, y_sb[:, dc, :], y_sb[:, dc, :])
                nc.tensor.matmul(
                    lps[0:1, :], lhsT=ones128, rhs=y_sb[:, dc, :],
                    start=(dc == 0), stop=(dc == KC - 1),
                    tile_position=(0, 0),
                )
                nc.tensor.matmul(
                    lps[32:33, :], lhsT=ones128, rhs=sq,
                    start=(dc == 0), stop=(dc == KC - 1),
                    tile_position=(0, 32),
                )
            o_sb = ffy.tile([128, KC, QT], BF, tag="o", name="o")
            engs = [nc.sync, nc.scalar]
            _emit_ln(
                tc, o_sb, y_sb, gb2_sb, ones128, eps_sb, ones_row,
                lnb, lnsq, pls, pub,
                out_dma=lambda c: engs[c % 2].dma_start(
                    outT[:, c, t0 : t0 + QT], o_sb[:, c, :]
                ),
                stats_ps=lps,
            )

        # fc2_ln2(0) is emitted BEFORE ln1(1): ln1(1) blocks on the second
        # ReduceScatter, and the strict-FIFO engine queues would stall all
        # of chunk-0's remaining work behind that wait
        ln1(0)
        fc(0)
        fc2_ln2(0, mid_cb=lambda: ln1(1))
        fc(1)
        fc2_ln2(1)


def _emit_ln(tc, out_sb, y_sb, gb_sb, ones128, eps_sb, ones_row, bpool,
             sqpool, pspool, bcpool, out_dma=None, stats_ps=None):
    """out = gb + (y - mean(y)) * rsqrt(var(y) + eps), feature-major bf16.

    y_sb: [128, KC, QT]. mean/var run over the full feature dim
    (partitions x KC chunks) via column-tiled ones-matmuls on the PE
    (sum at partition 0, sum-of-squares at partition 32 -> concurrent).
    The apply is fused as out = (y*RS + gb) - C with C = broadcast(u*rstd).
    """
    nc = tc.nc
    w = QT
    if stats_ps is not None:
        # caller already accumulated sum / sum-of-squares (inline with
        # its matmul stream); only the chain + apply remain here
        ps = stats_ps
    else:
        ps = pspool.tile([64, w], FP, tag="s12")
        for c in range(KC):
            sq = sqpool.tile([128, w], BF, tag="sq")
            nc.vector.tensor_mul(sq, y_sb[:, c, :], y_sb[:, c, :])
            nc.tensor.matmul(
                ps[0:1, :],
                lhsT=ones128,
                rhs=y_sb[:, c, :],
                start=(c == 0),
                stop=(c == KC - 1),
                tile_position=(0, 0),
            )
            nc.tensor.matmul(
                ps[32:33, :],
                lhsT=ones128,
                rhs=sq,
                start=(c == 0),
                stop=(c == KC - 1),
                tile_position=(0, 32),
            )
    u = bpool.tile([1, w], FP, tag="u")
    m2 = bpool.tile([1, w], FP, tag="m2")
    nc.scalar.activation(out=u, in_=ps[0:1, :], func=AF.Copy, scale=1.0 / D)
    nc.scalar.activation(out=m2, in_=ps[32:33, :], func=AF.Copy, scale=1.0 / D)
    var = bpool.tile([1, w], FP, tag="var")
    nc.vector.tensor_mul(var, u, u)
    nc.vector.tensor_tensor(out=var, in0=m2, in1=var, op=OP.subtract)
    sd = bpool.tile([1, w], FP, tag="sd")
    nc.scalar.activation(out=sd, in_=var, func=AF.Sqrt, bias=eps_sb)
    rstd = bpool.tile([1, w], BF, tag="rstd")
    _recip_fast(nc, rstd, sd)
    crow = bpool.tile([1, w], BF, tag="crow")
    nc.vector.tensor_mul(crow, u, rstd)
    RS = bcpool.tile([128, w], FP, tag="RS")
    CB = bcpool.tile([128, w], FP, tag="CB")
    nc.tensor.matmul(RS, lhsT=ones_row, rhs=rstd, start=True, stop=True)
    nc.tensor.matmul(CB, lhsT=ones_row, rhs=crow, start=True, stop=True)
    for c in range(KC):
        t = sqpool.tile([128, w], BF, tag="t")
        nc.vector.tensor_mul(t, y_sb[:, c, :], RS)
        nc.vector.scalar_tensor_tensor(
            out=out_sb[:, c, :],
            in0=t,
            scalar=gb_sb[:, c : c + 1],
            in1=CB,
            op0=OP.add,
            op1=OP.subtract,
        )
        if out_dma is not None:
            out_dma(c)


# ------------------------------------------------------------------ host side

def _pm(a2d):
    """[D_any, N] -> partition-major [128, D_any//128, N] (f = c*128 + p)."""
    d, n = a2d.shape
    return np.ascontiguousarray(a2d.reshape(d // 128, 128, n).transpose(1, 0, 2))


def _pmb(vec):
    """[D_any] -> [128, D_any//128] fp32 bias layout."""
    return np.ascontiguousarray(vec.reshape(-1, 128).T).astype(np.float32)


def shard_inputs(inputs):
    """Full inputs -> 8 per-core input dicts (numpy, partition-major bf16)."""
    bf = ml_dtypes.bfloat16
    f8 = ml_dtypes.float8_e4m3
    FS_NP = np.float32(FS)
    x = np.asarray(inputs["x"], np.float32)                        # [B, S, D]
    w_attn = np.asarray(inputs["w_attn"], np.float32)              # [D, 3D]
    b_attn = np.asarray(inputs["b_attn"], np.float32)
    w_proj = np.asarray(inputs["w_proj"], np.float32)
    b_proj = np.asarray(inputs["b_proj"], np.float32)
    w_fc = np.asarray(inputs["w_fc"], np.float32)
    b_fc = np.asarray(inputs["b_fc"], np.float32)
    w_fc2 = np.asarray(inputs["w_fc2"], np.float32)
    b_fc2 = np.asarray(inputs["b_fc2"], np.float32)
    gb1 = (np.asarray(inputs["ln1_g"]) + np.asarray(inputs["ln1_b"])).astype(
        np.float32
    )
    gb2 = (np.asarray(inputs["ln2_g"]) + np.asarray(inputs["ln2_b"])).astype(
        np.float32
    )
    # mask[p, r, t] = 1.0 if (128r + p) <= t else 0.0
    ki = (np.arange(DQT * 128).reshape(DQT, 128)).T[:, :, None]    # [128, DQT, 1]
    mask = (ki <= np.arange(QT)[None, None, :]).astype(bf)

    in_maps = []
    for c in range(N_CORES):
        b, p = c // 2, c % 2
        f0 = p * FH                      # first owned q/k/v feature
        xT = x[b].T                                              # [D, S]
        xhT = x[b, p * HALF : (p + 1) * HALF].T                  # [D, HALF]
        wqk = np.concatenate(
            [w_attn[:, f0 : f0 + FH], w_attn[:, D + f0 : D + f0 + FH]], axis=1
        )
        bqk = np.concatenate([b_attn[f0 : f0 + FH], b_attn[D + f0 : D + f0 + FH]])
        wv_ = w_attn[:, 2 * D + f0 : 2 * D + f0 + FH]
        bv_ = b_attn[2 * D + f0 : 2 * D + f0 + FH]
        wpr_ = w_proj[f0 : f0 + FH, :]
        in_maps.append(
            dict(
                xpm=_pm(xT).astype(bf),
                xh=_pm(xhT).astype(bf),
                wqk=_pm(wqk).astype(bf),
                bqk=_pmb(bqk),
                wv=_pm(wv_).astype(bf),
                bv=np.broadcast_to(
                    bv_.reshape(HL, DH).astype(bf), (128, HL, DH)
                ).copy(),
                wpr=_pm(wpr_).astype(bf),
                bpr=_pmb(b_proj),
                gb1=_pmb(gb1),
                gb2=_pmb(gb2),
                # fc1 weights: x16 into fp8-e4m3 (clip at TRN's +-240 max);
                # its rhs is n - gb1, so fold gb1@w_fc into the bias
                wfc=_pm(np.clip(w_fc * FS_NP, -240, 240)).astype(f8),
                bfc=_pmb(b_fc + gb1 @ w_fc),
                wfc2=_pm(w_fc2).astype(bf),
                bfc2=_pmb(b_fc2),
                mask=np.ascontiguousarray(mask),
            )
        )
    return in_maps


_IN_SPECS = dict(
    xpm=((128, KC, S), BF),
    xh=((128, KC, HALF), BF),
    wqk=((128, KC, 2 * FH), BF),
    bqk=((128, 2 * QC), FP),
    wv=((128, KC, FH), BF),
    bv=((128, HL, DH), BF),
    wpr=((128, QC, D), BF),
    bpr=((128, KC), FP),
    gb1=((128, KC), FP),
    gb2=((128, KC), FP),
    wfc=((128, KC, F), F8),
    bfc=((128, FC), FP),
    wfc2=((128, FC, D), BF),
    bfc2=((128, KC), FP),
    mask=((128, DQT, QT), BF),
)


def build_module():
    nc = bacc.Bacc(
        "TRN2", target_bir_lowering=False, debug=False, num_devices=N_CORES
    )
    ins = {
        name: nc.dram_tensor(name, list(shape), dt, kind="ExternalInput").ap()
        for name, (shape, dt) in _IN_SPECS.items()
    }
    outs = {
        "outT": nc.dram_tensor(
            "outT", [128, KC, HALF], BF, kind="ExternalOutput"
        ).ap(),
    }
    with tile.TileContext(nc) as tc:
        emit_block(tc, outs, ins)
    nc.compile()
    return nc


def unshard_output(results):
    """Per-core outT [128, KC, HALF] fp32 -> full [B, S, D]."""
    y = np.empty((B, S, D), np.float32)
    for c in range(N_CORES):
        b, p = c // 2, c % 2
        o = np.asarray(results[c]["outT"], np.float32)   # [128, KC, HALF]
        feat_major = o.transpose(1, 0, 2).reshape(D, HALF)
        y[b, p * HALF : (p + 1) * HALF, :] = feat_major.T
    return y


def kernel(**inputs):
    nc = build_module()
    in_maps = shard_inputs(inputs)
    res = run_bass_kernel_spmd(nc, in_maps, core_ids=list(range(N_CORES)))
    return unshard_output(res.results)



# revision 100
# speedup vs baseline: 1.0707x; 1.0149x over previous
"""Trainium2 Bass kernel for a GPT-2 style transformer block (B=4, S=2048, D=768).

Sharding (8 NeuronCores, one SPMD program):
  core c = (b, p): b = c // 2 (batch), p = c % 2 (tensor-parallel rank).
  - Attention is head-split: p=0 owns heads 0..5, p=1 owns heads 6..11,
    each over the FULL sequence of its batch.
  - c_attn / c_proj are computed only for the owned heads; the partial
    c_proj outputs are summed + token-scattered with a ReduceScatter over
    core pairs [[0,1],[2,3],[4,5],[6,7]] (bf16 payload).
  - LN1 / FFN / LN2 are token-split: p owns tokens [p*1024, (p+1)*1024).

All matmuls run in bf16 (fp32 PSUM accumulation). All DRAM tensors are
pre-shuffled host-side into partition-major [128, ...] layouts so DMAs are
contiguous per partition. Scores matmuls pack 2 heads via PE row tiling
(K=64 each); LN mean/var matmuls pack via PE column tiling. Softmax
denominators come free from a ones-column appended to V (M=65 AV matmuls);
reciprocals use the fast custom-DVE approximation.
"""

import numpy as np
import ml_dtypes

import concourse.bass as bass
import concourse.mybir as mybir
import concourse.tile as tile
from concourse import bacc
from concourse.bass_utils import run_bass_kernel_spmd

# ---------------------------------------------------------------- constants
B = 4
S = 2048
D = 768
H = 12
DH = 64
F = 3072
EPS = 1e-5

N_CORES = 8
HL = H // 2            # heads per core (6)
FH = HL * DH           # per-core attention feature width (384)
KC = D // 128          # contraction chunks over D (6)
QC = FH // 128         # feature chunks for per-core q or k (3)
FC = F // 128          # fc feature chunks (24)
QT = 512               # attention q-tile width
GQ = S // QT           # q tiles over full sequence (4)
DQT = QT // 128        # k-blocks per q tile width (4)
NKT = S // 128         # k-blocks over full sequence (16)
HALF = S // 2          # tokens owned per core for FFN/LN (1024)
TT = HALF // QT        # 512-token tiles per half (2)

FP = mybir.dt.float32
FPR = mybir.dt.float32r
BF = mybir.dt.bfloat16
F8 = mybir.dt.float8e4
DR = mybir.MatmulPerfMode.DoubleRow
FS = 16.0                # host-side scale on fp8 fc weights (dynamic range)

AF = mybir.ActivationFunctionType
OP = mybir.AluOpType


def _recip_fast(nc, out, in_):
    """out = approx 1/in_ (custom DVE op); out dtype may be bf16 (rounds
    on writeback), in_ must be fp32/PSUM-fp32 (bit-trick seed needs fp32
    layout)."""
    from concourse.dve_ops import RECIP_APPROX_FAST_CONSTS, RECIPROCAL_APPROX_FAST

    c = RECIP_APPROX_FAST_CONSTS
    return nc.vector._custom_dve(
        RECIPROCAL_APPROX_FAST, out=out, in0=in_, s0=c["s0"], s1=c["s1"],
        imm2=c["imm2"],
    )


def emit_block(tc, outs, ins):
    nc = tc.nc
    with nc.allow_low_precision(reason="bf16 matmul pipeline by design"):
        _emit_block_inner(tc, outs, ins)


def _emit_block_inner(tc, outs, ins):
    nc = tc.nc
    outT = outs["outT"]

    xpm = ins["xpm"]             # [128, KC, S] bf16
    xh = ins["xh"]               # [128, KC, HALF] bf16
    wqk = ins["wqk"]             # [128, KC, 2*FH] bf16
    bqk = ins["bqk"]             # [128, 2*QC] fp32
    wv = ins["wv"]               # [128, KC, FH] bf16
    bv = ins["bv"]               # [128, HL, DH] bf16 (broadcast materialized)
    wpr = ins["wpr"]             # [128, QC, D] bf16
    bpr = ins["bpr"]             # [128, KC] fp32
    gb1 = ins["gb1"]             # [128, KC] fp32
    gb2 = ins["gb2"]             # [128, KC] fp32
    wfc = ins["wfc"]             # [128, KC, F] bf16
    bfc = ins["bfc"]             # [128, FC] fp32
    wfc2 = ins["wfc2"]           # [128, FC, D] bf16
    bfc2 = ins["bfc2"]           # [128, KC] fp32
    mask = ins["mask"]           # [128, DQT, QT] bf16: mask[p, r, t] = (128r+p) <= t

    with (
        tc.tile_pool(name="const", bufs=1) as const,
        tc.tile_pool(name="dram", bufs=1, space="DRAM") as dram,
    ):
        # ---------------- constants
        # phase-1 consts go on the scalar HWDGE queue (cheap fixed cost);
        # FFN consts are deferred to ffn_loads() -- putting them on the
        # gpsimd SWDGE queue up front costs ~2us fixed EACH and delays the
        # attention weight loads behind them
        mask_sb = const.tile([128, DQT, QT], BF)
        bqk_sb = const.tile([128, 2 * QC], FP)
        nc.scalar.dma_start(bqk_sb, bqk)
        bv_sb = const.tile([128, HL, DH], BF)
        nc.scalar.dma_start(bv_sb, bv)
        bpr_sb = const.tile([128, KC], FP)
        gb1_sb = const.tile([128, KC], FP)
        gb2_sb = const.tile([128, KC], FP)
        bfc_sb = const.tile([128, FC], FP)
        bfc2_sb = const.tile([128, KC], FP)
        ones128 = const.tile([128, 1], BF)
        nc.vector.memset(ones128, 1.0)
        ones_row = const.tile([1, 128], BF)
        nc.vector.memset(ones_row, 1.0)
        eps_sb = const.tile([1, 1], FP)
        nc.vector.memset(eps_sb, EPS)
        # mask load is issued later (gpsimd queue, behind the attention
        # weights) so phase-1's x/wqk loads aren't delayed behind it

        # one DRAM tile per chunk so chunk-0 consumers don't falsely
        # depend on chunk-1's collective (dep tracking is per-tile)
        a_bounce = [
            dram.tile([2, 128, KC, QT], BF, tag=f"ab{c}", name=f"ab{c}")
            for c in range(TT)
        ]
        rs_out = [
            dram.tile([128, KC, QT], BF, tag=f"rs{c}", name=f"rs{c}")
            for c in range(TT)
        ]

        with tc.tile_pool(name="ffw", bufs=1) as ffw:
            # fc1 weights fp8 (x16 host-scaled, DoubleRow contracts 256
            # features per MM); fc2 stays bf16 -- quantizing both matmuls
            # pushes the output past the accuracy budget
            wfc_sb = ffw.tile([128, KC, F], F8)
            wfc2_sb = ffw.tile([128, FC, D], BF)

            with tc.tile_pool(name="nt", bufs=1) as npool:
                nT_sb = npool.tile([128, KC, HALF], BF)
                n8_sb = npool.tile([128, KC, HALF], F8)
                xh_sb = npool.tile([128, KC, HALF], BF)
                rs_sbs = [
                    npool.tile([128, KC, QT], BF, tag=f"rsb{i}", name=f"rsb{i}")
                    for i in range(TT)
                ]
                def ffn_loads():
                    nc.scalar.dma_start(bpr_sb, ins["bpr"])
                    nc.scalar.dma_start(gb1_sb, ins["gb1"])
                    nc.scalar.dma_start(gb2_sb, ins["gb2"])
                    nc.scalar.dma_start(bfc_sb, ins["bfc"])
                    nc.scalar.dma_start(bfc2_sb, ins["bfc2"])
                    nc.scalar.dma_start(xh_sb, xh)
                    nc.scalar.dma_start(wfc_sb, ins["wfc"])
                    nc.scalar.dma_start(wfc2_sb, ins["wfc2"])

                _emit_qkv_attention(
                    tc, ins, a_bounce, rs_out, rs_sbs, mask_sb, bqk_sb,
                    bv_sb, ones_row, ffn_loads,
                )

                # ====== phases 4+5: LN1 -> FFN -> LN2 per 512-token chunk ==
                _emit_ffn(
                    tc, outT, nT_sb, n8_sb, xh_sb, rs_sbs, wfc_sb, wfc2_sb,
                    bpr_sb, gb1_sb, gb2_sb, bfc_sb, bfc2_sb, ones128, eps_sb,
                    ones_row,
                )


def _emit_qkv_attention(tc, ins, a_bounce, rs_out, rs_sbs, mask_sb, bqk_sb,
                        bv_sb, ones_row, ffn_loads):
    nc = tc.nc
    xpm = ins["xpm"]
    wqk = ins["wqk"]
    wv = ins["wv"]
    wpr = ins["wpr"]

    with tc.tile_pool(name="kqv", bufs=1) as kqv:
        # persistent attention activations (bf16)
        kT_sb = kqv.tile([128, QC, S], BF)            # k, feature-major
        qT_sb = kqv.tile([128, QC, S], BF)            # q, feature-major
        v_sb = kqv.tile([128, NKT, HL, DH + 1], BF)   # v token-major + ones
        nc.vector.memset(v_sb[:, :, :, DH : DH + 1], 1.0)

        # ================ phase 1: qkv projections =====================
        with (
            tc.tile_pool(name="p1", bufs=2) as p1,
            tc.tile_pool(name="p1w", bufs=1) as p1w,
            tc.tile_pool(name="psqk", bufs=4, space="PSUM") as psqk,
            tc.tile_pool(name="psv", bufs=2, space="PSUM") as psv,
        ):
            # all phase-1 loads issued up front as few, large DMAs (HWDGE
            # fixed cost is ~0.6us EACH): wqk split across the two HWDGE
            # queues, x split 3 ways (sync/scalar/gpsimd) so the first-MM
            # gate isn't one queue's serial drain; mask goes LAST on
            # gpsimd (first needed only when attention starts). FFN
            # weights only start after these (ffn_loads).
            KH = KC // 2
            wqk_sb = p1w.tile([128, KC, 2 * FH], BF)
            nc.sync.dma_start(wqk_sb[:, 0:KH, :], wqk[:, 0:KH, :])
            nc.scalar.dma_start(wqk_sb[:, KH:, :], wqk[:, KH:, :])
            wv_sb = p1w.tile([128, KC, FH], BF)
            nc.gpsimd.dma_start(wv_sb, wv)
            x_ts = []
            for half in range(2):
                t0 = half * HALF
                x_t = p1.tile([128, KC, HALF], BF, tag="xT")
                x_ts.append(x_t)
                nc.sync.dma_start(
                    x_t[:, 0:2, :], xpm[:, 0:2, t0 : t0 + HALF]
                )
                nc.scalar.dma_start(
                    x_t[:, 2:4, :], xpm[:, 2:4, t0 : t0 + HALF]
                )
                nc.gpsimd.dma_start(
                    x_t[:, 4:6, :], xpm[:, 4:6, t0 : t0 + HALF]
                )
            nc.gpsimd.dma_start(mask_sb, ins["mask"])

            for half in range(2):
                t0 = half * HALF
                x_t = x_ts[half]
                # q / k feature-major: out[feat_chunk, tokens]
                for fc in range(2 * QC):
                    for ttt in range(TT):
                        ps = psqk.tile([128, QT], FP)
                        for k in range(KC):
                            nc.tensor.matmul(
                                ps,
                                lhsT=wqk_sb[:, k, 128 * fc : 128 * fc + 128],
                                rhs=x_t[:, k, QT * ttt : QT * ttt + QT],
                                start=(k == 0),
                                stop=(k == KC - 1),
                            )
                        dst = qT_sb if fc < QC else kT_sb
                        cc = fc if fc < QC else fc - QC
                        nc.vector.tensor_scalar_add(
                            dst[:, cc, t0 + QT * ttt : t0 + QT * ttt + QT],
                            ps,
                            bqk_sb[:, fc : fc + 1],
                        )
                # v token-major: out[token_chunk, v features]
                for tcc in range(HALF // 128):
                    ps = psv.tile([128, FH], FP)
                    for k in range(KC):
                        nc.tensor.matmul(
                            ps,
                            lhsT=x_t[:, k, 128 * tcc : 128 * tcc + 128],
                            rhs=wv_sb[:, k, :],
                            start=(k == 0),
                            stop=(k == KC - 1),
                        )
                    tok = half * (HALF // 128) + tcc
                    nc.vector.tensor_add(
                        v_sb[:, tok, :, 0:DH],
                        ps.rearrange("p (h d) -> p h d", h=HL),
                        bv_sb,
                    )

        # FFN weights + residual stream: issued only now so phase-1's
        # x/wqk loads aren't stuck behind 9.4MB of FFN weights
        ffn_loads()
        # ============ phase 2+3: attention, normalize, c_proj ==========
        _emit_attention(
            tc, a_bounce, rs_out, rs_sbs, kT_sb, qT_sb, v_sb, wpr, mask_sb,
            ones_row,
        )


def _emit_attention(tc, a_bounce, rs_out, rs_sbs, kT_sb, qT_sb, v_sb, wpr,
                    mask_sb, ones_row):
    nc = tc.nc
    with (
        tc.tile_pool(name="att", bufs=1) as att,
        tc.tile_pool(name="attR", bufs=2) as attR,
        tc.tile_pool(name="pss", bufs=2, space="PSUM") as pss,
        tc.tile_pool(name="psav", bufs=1, space="PSUM") as psav,
        tc.tile_pool(name="pscp", bufs=2, space="PSUM") as pscp,
    ):
        aT_sb = att.tile([128, QC, S], BF)    # attention out, feature-major
        wpr_sb = att.tile([128, QC, D], BF)
        nc.sync.dma_start(wpr_sb, wpr)
        # manual exp ring: masked-out columns of diagonal blocks are
        # never re-exp'd; the mask multiply zeroes them against stale
        # (finite) data, seeded by this one-time memset.
        # 4 ring slots: with 3, q-tiles whose block count is 1 mod 3
        # serialize the next head-pair's first exp behind this pair's last
        # AV via the slot-0 write-after-read
        exm = att.tile([128, 4, 2, QT], BF)
        nc.vector.memset(exm, 0.0)

        # the per-g tail (normalize + c_proj + ReduceScatter) is split into
        # ~10 small pieces drained ONE PER j-ITERATION of the next g's
        # attention loops: each piece's PE/DVE work hides in the slack
        # under the exp stream instead of stalling softmax ~14us per g
        tail_q = []

        def drain_piece():
            if tail_q:
                tail_q.pop(0)()

        for gi, g in enumerate([0, 2, 1, 3]):
            q0 = g * QT
            nkt = DQT * (g + 1)
            rec_sb = attR.tile([1, HL * QT], BF, tag="rec")
            def scores(m, j):
                ps = pss.tile([128, 2, QT], FP, tag="ss", name="ps")
                jd = j - DQT * g
                c0 = 128 * jd if jd > 0 else 0
                # 2 heads row-tiled (K=64 each) -> concurrent MMs;
                # diagonal blocks stream only the causally-live cols
                for i in range(2):
                    hr = i * 64
                    nc.tensor.matmul(
                        ps[:, i, c0:],
                        lhsT=kT_sb[
                            hr : hr + 64, m, 128 * j : 128 * j + 128
                        ],
                        rhs=qT_sb[hr : hr + 64, m, q0 + c0 : q0 + QT],
                        start=True,
                        stop=True,
                    )
                return ps

            ps = scores(0, 0)
            for m in range(QC):  # head pairs (2m, 2m+1)
                den_sb = attR.tile([1, 2, QT], FP, tag="den")
                pavs = [
                    psav.tile(
                        [DH + 1, QT], FP, tag=f"pav{i}", name=f"pav{i}"
                    )
                    for i in range(2)
                ]
                for j in range(nkt):
                    jd = j - DQT * g
                    c0 = 128 * jd if jd > 0 else 0
                    ex = exm[:, j % 4, :, :]
                    nc.scalar.activation(
                        out=ex[:, :, c0:],
                        in_=ps[:, :, c0:],
                        func=AF.Exp,
                    )
                    # software pipeline: the NEXT scores matmuls (next j,
                    # or the next head-pair's j=0) are emitted BEFORE this
                    # j's AV matmuls. The PE queue is strict FIFO and AV_j
                    # waits on exp_j -- anything behind AV_j serializes the
                    # loop at (exp+AV+scores) per step. Carrying the
                    # lookahead ACROSS head-pair boundaries keeps the exp
                    # stream fed through all 12 boundaries; the 4-slot exp
                    # ring makes slot 0 WAR-free there on every tile.
                    if j + 1 < nkt:
                        ps = scores(m, j + 1)
                    elif m + 1 < QC:
                        ps = scores(m + 1, 0)
                    if jd >= 0:
                        for i in range(2):
                            nc.vector.tensor_mul(
                                ex[:, i, :], ex[:, i, :], mask_sb[:, jd, :]
                            )
                    for i in range(2):
                        nc.tensor.matmul(
                            pavs[i],
                            lhsT=v_sb[:, j, 2 * m + i, :],
                            rhs=ex[:, i, :],
                            start=(j == 0),
                            stop=(j == nkt - 1),
                        )
                    drain_piece()
                # PSUM->SBUF copies stay off the scalar engine: scalar is
                # saturated by the softmax exp stream (the phase bottleneck)
                for i in range(2):
                    nc.vector.tensor_copy(
                        out=den_sb[:, i, :],
                        in_=pavs[i][DH : DH + 1, :],
                    )
                    nc.vector.tensor_copy(
                        out=aT_sb[64 * i : 64 * i + 64, m,
                                  q0 : q0 + QT],
                        in_=pavs[i][0:DH, :],
                    )
                _recip_fast(
                    nc,
                    rec_sb[:, 2 * m * QT : 2 * m * QT + 2 * QT],
                    den_sb.rearrange("p a t -> p (a t)"),
                )
            pieces = []
            sth = {}

            def norm(m, q0=q0, rec_sb=rec_sb):
                # normalize by softmax denominator (paired PE broadcast)
                rbt = pscp.tile([128, QT], FP, tag="cp", name="rbt")
                for i in range(2):
                    h = 2 * m + i
                    nc.tensor.matmul(
                        rbt[64 * i : 64 * i + 64, :],
                        lhsT=ones_row[:, 0:64],
                        rhs=rec_sb[:, h * QT : h * QT + QT],
                        start=True,
                        stop=True,
                        tile_position=(0, 64 * i),
                    )
                nc.vector.tensor_mul(
                    aT_sb[:, m, q0 : q0 + QT],
                    aT_sb[:, m, q0 : q0 + QT],
                    rbt,
                )

            def cproj(dc, q0=q0, sth=sth):
                # partial c_proj for this q tile (bf16 payload halves the
                # bounce DMA and the ReduceScatter wire)
                if dc == 0:
                    sth["st"] = attR.tile(
                        [128, KC, QT], BF, tag="prst", name="prst"
                    )
                pt = pscp.tile([128, QT], FP, tag="cp", name="pt")
                for kc in range(QC):
                    nc.tensor.matmul(
                        pt,
                        lhsT=wpr_sb[:, kc, 128 * dc : 128 * dc + 128],
                        rhs=aT_sb[:, kc, q0 : q0 + QT],
                        start=(kc == 0),
                        stop=(kc == QC - 1),
                    )
                # vector, not scalar: scalar's queue carries the exp
                # stream and a copy here would stall the next softmax
                nc.vector.tensor_copy(out=st_slice(sth, dc), in_=pt)

            def st_slice(sth, dc):
                return sth["st"][:, dc, :]

            def fin(g=g, gi=gi, sth=sth):
                ch, th = g % 2, g // 2
                nc.sync.dma_start(a_bounce[ch][th], sth["st"])
                if gi in (1, 3):
                    # both halves of chunk ch are now written -> exchange,
                    # then pull the result into SBUF immediately (gpsimd
                    # queue: stalls only this queue until the RS lands)
                    nc.gpsimd.collective_compute(
                        "ReduceScatter",
                        OP.add,
                        replica_groups=[[0, 1], [2, 3], [4, 5], [6, 7]],
                        ins=[a_bounce[ch].opt()],
                        outs=[rs_out[ch].opt()],
                    )
                    nc.gpsimd.dma_start(rs_sbs[ch], rs_out[ch])

            for m in range(QC):
                pieces.append(lambda m=m: norm(m))
            for dc in range(KC):
                pieces.append(lambda dc=dc: cproj(dc))
            pieces.append(fin)

            if gi == 3:
                for p in pieces:
                    p()
            else:
                tail_q.extend(pieces)
        while tail_q:
            tail_q.pop(0)()


def _emit_ffn(tc, outT, nT_sb, n8_sb, xh_sb, rs_sbs, wfc_sb, wfc2_sb, bpr_sb,
              gb1_sb, gb2_sb, bfc_sb, bfc2_sb, ones128, eps_sb, ones_row):
    nc = tc.nc
    with (
        tc.tile_pool(name="ln", bufs=2) as ln,
        tc.tile_pool(name="lnb", bufs=2) as lnb,
        tc.tile_pool(name="lnsq", bufs=2) as lnsq,
        tc.tile_pool(name="ffn", bufs=1) as ffn,
        tc.tile_pool(name="ffy", bufs=1) as ffy,
        tc.tile_pool(name="pls", bufs=2, space="PSUM") as pls,
        tc.tile_pool(name="pub", bufs=1, space="PSUM") as pub,
        tc.tile_pool(name="psfc", bufs=2, space="PSUM") as psfc,
        tc.tile_pool(name="psf2", bufs=2, space="PSUM") as psf2,
    ):
        hT_sbs = {}

        def ln1(ht):
            t0 = ht * QT
            rs_sb = rs_sbs[ht]
            y1_sb = ln.tile([128, KC, QT], BF, tag="y1", name="y1")
            for c in range(KC):
                nc.vector.scalar_tensor_tensor(
                    out=y1_sb[:, c, :],
                    in0=rs_sb[:, c, :],
                    scalar=bpr_sb[:, c : c + 1],
                    in1=xh_sb[:, c, t0 : t0 + QT],
                    op0=OP.add,
                    op1=OP.add,
                )
            # LN1 output is also needed as fp8 (the fc1 DoubleRow rhs),
            # DC-shifted by gb1 so the fp8 grid covers the +-1-ish
            # normalized part (the gb1@w_fc term is folded into fc1's bias
            # host-side). Scalar is idle here (the exp stream is done).
            _emit_ln(
                tc, nT_sb[:, :, t0 : t0 + QT], y1_sb, gb1_sb, ones128, eps_sb,
                ones_row, lnb, lnsq, pls, pub,
                out_dma=lambda c: nc.vector.tensor_scalar_sub(
                    n8_sb[:, c, t0 : t0 + QT],
                    nT_sb[:, c, t0 : t0 + QT],
                    gb1_sb[:, c : c + 1],
                ),
            )

        def fc(ht):
            t0 = ht * QT
            hT_sb = ffn.tile([128, FC, QT], BF, tag="hT", name="hT")
            hT_sbs[ht] = hT_sb
            for fci in range(FC):
                ps = psfc.tile([128, QT], FP)
                for kp in range(KC // 2):
                    nc.tensor.matmul(
                        ps,
                        lhsT=wfc_sb[:, 2 * kp : 2 * kp + 2,
                                    128 * fci : 128 * fci + 128],
                        rhs=n8_sb[:, 2 * kp : 2 * kp + 2, t0 : t0 + QT],
                        start=(kp == 0),
                        stop=(kp == KC // 2 - 1),
                        perf_mode=DR,
                    )
                # ps = FS*((n-gb1)@w_fc); bfc carries b_fc + gb1@w_fc
                nc.scalar.activation(
                    out=hT_sb[:, fci, :],
                    in_=ps,
                    func=AF.Relu,
                    bias=bfc_sb[:, fci : fci + 1],
                    scale=1.0 / FS,
                )

        def fc2_ln2(ht, mid_cb=None):
            t0 = ht * QT
            nT_c = nT_sb[:, :, t0 : t0 + QT]
            hT_sb = hT_sbs.pop(ht)
            y_sb = ffy.tile([128, KC, QT], BF, tag="y", name="y")
            # LN2 stats accumulate inline as each y chunk lands, so only
            # the short serial chain remains after the last fc2 group
            # (pls bufs=2: ln1(1) is injected mid-loop and must not share
            # this accumulator's bank)
            lps = pls.tile([64, QT], FP, tag="s12", name="lps")
            for dc in range(KC):
                if dc == 3 and mid_cb is not None:
                    # injected here so its serial LN chain overlaps the
                    # rest of this chunk's fc2 matmul stream
                    mid_cb()
                ps2 = psf2.tile([128, QT], FP)
                for k in range(FC):
                    nc.tensor.matmul(
                        ps2,
                        lhsT=wfc2_sb[:, k, 128 * dc : 128 * dc + 128],
                        rhs=hT_sb[:, k, :],
                        start=(k == 0),
                        stop=(k == FC - 1),
                    )
                nc.vector.scalar_tensor_tensor(
                    out=y_sb[:, dc, :],
                    in0=ps2,
                    scalar=bfc2_sb[:, dc : dc + 1],
                    in1=nT_c[:, dc, :],
                    op0=OP.add,
                    op1=OP.add,
                )
                sq = lnsq.tile([128, QT], BF, tag="sq")
                nc.vector.tensor_mul(sq, y_sb[:, dc, :], y_sb[:, dc, :])
                nc.tensor.matmul(
                    lps[0:1, :], lhsT=ones128, rhs=y_sb[:, dc, :],
                    start=(dc == 0), stop=(dc == KC - 1),
                    tile_position=(0, 0),
                )
                nc.tensor.matmul(
                    lps[32:33, :], lhsT=ones128, rhs=sq,
                    start=(dc == 0), stop=(dc == KC - 1),
                    tile_position=(0, 32),
                )
            o_sb = ffy.tile([128, KC, QT], BF, tag="o", name="o")
            engs = [nc.sync, nc.scalar]
            _emit_ln(
                tc, o_sb, y_sb, gb2_sb, ones128, eps_sb, ones_row,
                lnb, lnsq, pls, pub,
                out_dma=lambda c: engs[c % 2].dma_start(
                    outT[:, c, t0 : t0 + QT], o_sb[:, c, :]
                ),
                stats_ps=lps,
            )

        # fc2_ln2(0) is emitted BEFORE ln1(1): ln1(1) blocks on the second
        # ReduceScatter, and the strict-FIFO engine queues would stall all
        # of chunk-0's remaining work behind that wait
        ln1(0)
        fc(0)
        fc2_ln2(0, mid_cb=lambda: ln1(1))
        fc(1)
        fc2_ln2(1)


def _emit_ln(tc, out_sb, y_sb, gb_sb, ones128, eps_sb, ones_row, bpool,
             sqpool, pspool, bcpool, out_dma=None, stats_ps=None):
    """out = gb + (y - mean(y)) * rsqrt(var(y) + eps), feature-major bf16.

    y_sb: [128, KC, QT]. mean/var run over the full feature dim
    (partitions x KC chunks) via column-tiled ones-matmuls on the PE
    (sum at partition 0, sum-of-squares at partition 32 -> concurrent).
    The apply is fused as out = (y*RS + gb) - C with C = broadcast(u*rstd).
    """
    nc = tc.nc
    w = QT
    if stats_ps is not None:
        # caller already accumulated sum / sum-of-squares (inline with
        # its matmul stream); only the chain + apply remain here
        ps = stats_ps
    else:
        ps = pspool.tile([64, w], FP, tag="s12")
        for c in range(KC):
            sq = sqpool.tile([128, w], BF, tag="sq")
            nc.vector.tensor_mul(sq, y_sb[:, c, :], y_sb[:, c, :])
            nc.tensor.matmul(
                ps[0:1, :],
                lhsT=ones128,
                rhs=y_sb[:, c, :],
                start=(c == 0),
                stop=(c == KC - 1),
                tile_position=(0, 0),
            )
            nc.tensor.matmul(
                ps[32:33, :],
                lhsT=ones128,
                rhs=sq,
                start=(c == 0),
                stop=(c == KC - 1),
                tile_position=(0, 32),
            )
    u = bpool.tile([1, w], FP, tag="u")
    m2 = bpool.tile([1, w], FP, tag="m2")
    nc.scalar.activation(out=u, in_=ps[0:1, :], func=AF.Copy, scale=1.0 / D)
    nc.scalar.activation(out=m2, in_=ps[32:33, :], func=AF.Copy, scale=1.0 / D)
    var = bpool.tile([1, w], FP, tag="var")
    nc.vector.tensor_mul(var, u, u)
    nc.vector.tensor_tensor(out=var, in0=m2, in1=var, op=OP.subtract)
    sd = bpool.tile([1, w], FP, tag="sd")
    nc.scalar.activation(out=sd, in_=var, func=AF.Sqrt, bias=eps_sb)
    rstd = bpool.tile([1, w], BF, tag="rstd")
    _recip_fast(nc, rstd, sd)
    crow = bpool.tile([1, w], BF, tag="crow")
    nc.vector.tensor_mul(crow, u, rstd)
    RS = bcpool.tile([128, w], FP, tag="RS")
    CB = bcpool.tile([128, w], FP, tag="CB")
    nc.tensor.matmul(RS, lhsT=ones_row, rhs=rstd, start=True, stop=True)
    nc.tensor.matmul(CB, lhsT=ones_row, rhs=crow, start=True, stop=True)
    for c in range(KC):
        t = sqpool.tile([128, w], BF, tag="t")
        nc.vector.tensor_mul(t, y_sb[:, c, :], RS)
        nc.vector.scalar_tensor_tensor(
            out=out_sb[:, c, :],
            in0=t,
            scalar=gb_sb[:, c : c + 1],
            in1=CB,
            op0=OP.add,
            op1=OP.subtract,
        )
        if out_dma is not None:
            out_dma(c)


# ------------------------------------------------------------------ host side

def _pm(a2d):
    """[D_any, N] -> partition-major [128, D_any//128, N] (f = c*128 + p)."""
    d, n = a2d.shape
    return np.ascontiguousarray(a2d.reshape(d // 128, 128, n).transpose(1, 0, 2))


def _pmb(vec):
    """[D_any] -> [128, D_any//128] fp32 bias layout."""
    return np.ascontiguousarray(vec.reshape(-1, 128).T).astype(np.float32)


def shard_inputs(inputs):
    """Full inputs -> 8 per-core input dicts (numpy, partition-major bf16)."""
    bf = ml_dtypes.bfloat16
    f8 = ml_dtypes.float8_e4m3
    FS_NP = np.float32(FS)
    x = np.asarray(inputs["x"], np.float32)                        # [B, S, D]
    w_attn = np.asarray(inputs["w_attn"], np.float32)              # [D, 3D]
    b_attn = np.asarray(inputs["b_attn"], np.float32)
    w_proj = np.asarray(inputs["w_proj"], np.float32)
    b_proj = np.asarray(inputs["b_proj"], np.float32)
    w_fc = np.asarray(inputs["w_fc"], np.float32)
    b_fc = np.asarray(inputs["b_fc"], np.float32)
    w_fc2 = np.asarray(inputs["w_fc2"], np.float32)
    b_fc2 = np.asarray(inputs["b_fc2"], np.float32)
    gb1 = (np.asarray(inputs["ln1_g"]) + np.asarray(inputs["ln1_b"])).astype(
        np.float32
    )
    gb2 = (np.asarray(inputs["ln2_g"]) + np.asarray(inputs["ln2_b"])).astype(
        np.float32
    )
    # mask[p, r, t] = 1.0 if (128r + p) <= t else 0.0
    ki = (np.arange(DQT * 128).reshape(DQT, 128)).T[:, :, None]    # [128, DQT, 1]
    mask = (ki <= np.arange(QT)[None, None, :]).astype(bf)

    in_maps = []
    for c in range(N_CORES):
        b, p = c // 2, c % 2
        f0 = p * FH                      # first owned q/k/v feature
        xT = x[b].T                                              # [D, S]
        xhT = x[b, p * HALF : (p + 1) * HALF].T                  # [D, HALF]
        wqk = np.concatenate(
            [w_attn[:, f0 : f0 + FH], w_attn[:, D + f0 : D + f0 + FH]], axis=1
        )
        bqk = np.concatenate([b_attn[f0 : f0 + FH], b_attn[D + f0 : D + f0 + FH]])
        wv_ = w_attn[:, 2 * D + f0 : 2 * D + f0 + FH]
        bv_ = b_attn[2 * D + f0 : 2 * D + f0 + FH]
        wpr_ = w_proj[f0 : f0 + FH, :]
        in_maps.append(
            dict(
                xpm=_pm(xT).astype(bf),
                xh=_pm(xhT).astype(bf),
                wqk=_pm(wqk).astype(bf),
                bqk=_pmb(bqk),
                wv=_pm(wv_).astype(bf),
                bv=np.broadcast_to(
                    bv_.reshape(HL, DH).astype(bf), (128, HL, DH)
                ).copy(),
                wpr=_pm(wpr_).astype(bf),
                bpr=_pmb(b_proj),
                gb1=_pmb(gb1),
                gb2=_pmb(gb2),
                # fc1 weights: x16 into fp8-e4m3 (clip at TRN's +-240 max);
                # its rhs is n - gb1, so fold gb1@w_fc into the bias
                wfc=_pm(np.clip(w_fc * FS_NP, -240, 240)).astype(f8),
                bfc=_pmb(b_fc + gb1 @ w_fc),
                wfc2=_pm(w_fc2).astype(bf),
                bfc2=_pmb(b_fc2),
                mask=np.ascontiguousarray(mask),
            )
        )
    return in_maps


_IN_SPECS = dict(
    xpm=((128, KC, S), BF),
    xh=((128, KC, HALF), BF),
    wqk=((128, KC, 2 * FH), BF),
    bqk=((128, 2 * QC), FP),
    wv=((128, KC, FH), BF),
    bv=((128, HL, DH), BF),
    wpr=((128, QC, D), BF),
    bpr=((128, KC), FP),
    gb1=((128, KC), FP),
    gb2=((128, KC), FP),
    wfc=((128, KC, F), F8),
    bfc=((128, FC), FP),
    wfc2=((128, FC, D), BF),
    bfc2=((128, KC), FP),
    mask=((128, DQT, QT), BF),
)


def build_module():
    nc = bacc.Bacc(
        "TRN2", target_bir_lowering=False, debug=False, num_devices=N_CORES
    )
    ins = {
        name: nc.dram_tensor(name, list(shape), dt, kind="ExternalInput").ap()
        for name, (shape, dt) in _IN_SPECS.items()
    }
    outs = {
        "outT": nc.dram_tensor(
            "outT", [128, KC, HALF], BF, kind="ExternalOutput"
        ).ap(),
    }
    with tile.TileContext(nc) as tc:
        emit_block(tc, outs, ins)
    nc.compile()
    return nc


def unshard_output(results):
    """Per-core outT [128, KC, HALF] fp32 -> full [B, S, D]."""
    y = np.empty((B, S, D), np.float32)
    for c in range(N_CORES):
        b, p = c // 2, c % 2
        o = np.asarray(results[c]["outT"], np.float32)   # [128, KC, HALF]
        feat_major = o.transpose(1, 0, 2).reshape(D, HALF)
        y[b, p * HALF : (p + 1) * HALF, :] = feat_major.T
    return y


def kernel(**inputs):
    nc = build_module()
    in_maps = shard_inputs(inputs)
    res = run_bass_kernel_spmd(nc, in_maps, core_ids=list(range(N_CORES)))
    return unshard_output(res.results)

